# revision 15
# baseline (speedup 1.0000x reference)
"""GAT (2-layer, 8-head then 1-head) on 8 Trainium2 NeuronCores.

v2 design: dst-shard nodes across 8 cores (12544-padded shards). Per layer,
each core builds a bf16 node table [npad, 128] = [msg(64) | Es | es | 0...]
where Es = exp(al_src), es = exp(0.2*al_src); AllGathered to all cores.
Attention uses the factorization
    exp(leaky(als+ald)) = max(Es*Ed, es*ed),   Ed = exp(al_dst), ed = exp(.2*al_dst)
so the per-edge work is two multiplies and a max; Ed/ed come from a per-window
resident table looked up on the PE via one-hot S^T matmuls (bf16 hi/lo pairs
accumulated in f32 PSUM for full precision).

Edges are dst-owned, grouped by (block of 7 windows, src-super-shard of 25088
rows, window) with core-common structure; h[src] rows are fetched with ONE
dma_gather per (block, super-shard) stream (~3.9k edges) to amortize the ~5us
SWDGE fixed cost. One-hot S matrices are built on DVE (st1 hits the 2x 16-bit
path against a constant iotacol_wide). Aggregation matmuls accumulate
[msg*alpha | alpha] into per-window PSUM slices packed 7-to-a-bank.
Self-loops are folded in as per-window SBUF-resident rhs computed during the
producer phase and added via one identity matmul per window. Layer 2
aggregates relu1 and applies W2 after aggregation; 1/denominator is applied
after the W2 matmul (row scalars commute).
"""
import sys
import numpy as np

sys.path.insert(0, "/opt/trn_rl_repo")
import ml_dtypes

BF = ml_dtypes.bfloat16

N = 100000
F_IN = 128
HID = 8
HEADS = 8
CLASSES = 40
NEG = 0.2
NC = 8

FULL_CFG = dict(
    ncores=8, nshard=12500, npad=12544, wb=7, ngsh=4, ni_max=3968,
    f_in=128, heads=8, hid=8, classes=40, neg=0.2,
)


def _ceil(a, b):
    return -(-a // b)


# ---------------------------------------------------------------- host prep
def prep_structure(src, dst, cfg):
    """Build core-common call/tile/matmul structure + per-core arrays.

    Returns (st, percore): st holds the shared program structure, percore[c]
    holds the packed meta (idx|dcol) and drow arrays for core c.
    """
    nc_, nsh, npad, wb = cfg["ncores"], cfg["nshard"], cfg["npad"], cfg["wb"]
    ngsh, NI = cfg["ngsh"], cfg["ni_max"]
    nwin = npad // 128                      # 98
    nblk = _ceil(nwin, wb)                  # 14
    assert nwin % wb == 0, "code assumes full blocks"
    gsz = (nc_ * npad) // ngsh              # 25088 rows per gather super-shard
    padrow = nsh                            # zero-ish row within super-shard

    core = dst // nsh
    dstl = dst % nsh
    g_row = (src // nsh) * npad + (src % nsh)
    gsh = g_row // gsz
    gloc = g_row % gsz
    win = dstl // 128
    blk = win // wb

    counts = np.zeros((nc_, nblk, ngsh, nwin), dtype=np.int64)
    np.add.at(counts, (core, blk, gsh, win), 1)
    common = counts.max(axis=0)             # [nblk, ngsh, nwin]
    for b in range(nblk):
        assert common[b, :, b * wb:(b + 1) * wb].min() >= 128, \
            "tile could span >2 windows"

    # per-core edge arrays sorted by (block, gshard, window)
    percore_edges = []
    for c in range(nc_):
        m = core == c
        key = (blk[m].astype(np.int64) * ngsh + gsh[m]) * nwin + win[m]
        o = np.argsort(key, kind="stable")
        percore_edges.append((key[o], gloc[m][o], dstl[m][o], win[m][o]))

    calls = []
    pc_meta = [[] for _ in range(nc_)]
    pc_drow = [[] for _ in range(nc_)]
    win_mms = {}                            # (b, w) -> list of mm ids
    MC = NI // 16 + 32                      # meta cols (idx | dcol-as-i16)

    for b in range(nblk):
        for s in range(ngsh):
            streams = []
            for c in range(nc_):
                kk, sl, dl, wn = percore_edges[c]
                segs = []
                for w in range(b * wb, (b + 1) * wb):
                    kval = (b * ngsh + s) * nwin + w
                    lo = np.searchsorted(kk, kval, "left")
                    hi = np.searchsorted(kk, kval, "right")
                    n_common = common[b, s, w]
                    seg_s = np.full(n_common, padrow, dtype=np.int64)
                    seg_w = np.full(n_common, w, dtype=np.int64)
                    seg_d = np.full(n_common, -1, dtype=np.int64)
                    seg_s[: hi - lo] = sl[lo:hi]
                    seg_w[: hi - lo] = wn[lo:hi]
                    seg_d[: hi - lo] = dl[lo:hi]
                    segs.append(np.stack([seg_s, seg_w, seg_d]))
                streams.append(np.concatenate(segs, axis=1))
            L = streams[0].shape[1]
            pos = 0
            while pos < L:
                ni_real = min(NI, L - pos)
                ni = _ceil(ni_real, 128) * 128
                ntile = ni // 128
                wseg = streams[0][1][pos:pos + ni_real]
                tiles = []
                for t in range(ntile):
                    a, z = t * 128, min((t + 1) * 128, ni_real)
                    if a < ni_real:
                        tw = wseg[a:z]
                        w1 = int(tw.min())
                        assert int(tw.max()) - w1 <= 1, "tile spans >2 windows"
                        straddle = int(tw.max()) > w1
                    else:
                        w1, straddle = int(wseg[-1]), False
                    tiles.append((w1, straddle))
                cid = len(calls)
                mms = []
                for t, (w1, straddle) in enumerate(tiles):
                    for k in ([0, 1] if straddle else [0]):
                        mm_id = (cid, t, k, w1 + k)
                        win_mms.setdefault((b, w1 + k), []).append(mm_id)
                        mms.append(mm_id)
                calls.append(dict(blk=b, shard=s, ni=ni, ntile=ntile,
                                  tiles=tiles, mms=mms))
                for c in range(nc_):
                    ss, ww, dd = streams[c]
                    sl_call = np.full(ni, padrow, dtype=np.int64)
                    rel_call = np.full(ni, 300.0, dtype=np.float64)
                    nreal = min(ni_real, L - pos)
                    sl_call[:nreal] = ss[pos:pos + nreal]
                    for t in range(ntile):
                        a, z = t * 128, min((t + 1) * 128, nreal)
                        if a >= nreal:
                            break
                        w1 = tiles[t][0]
                        dv = dd[pos + a:pos + z]
                        wv = ww[pos + a:pos + z]
                        rel = (wv - w1) * 128 + (dv - wv * 128)
                        rel = np.where(dv < 0, 300.0, rel)
                        rel_call[a:z] = rel
                    # meta: idx wrapped [16, ni/16] tiled x8, then dcol bf16 bits
                    meta = np.zeros((128, MC), np.int16)
                    iw = sl_call.reshape(ni // 16, 16).T.astype(np.int16)
                    meta[:, 0:ni // 16] = np.tile(iw, (8, 1))
                    dcol = rel_call.reshape(ntile, 128).T.astype(BF)
                    meta[:, NI // 16:NI // 16 + ntile] = dcol.view(np.int16)
                    pc_meta[c].append(meta)
                    pc_drow[c].append(rel_call.astype(BF))
                pos += ni_real

    # One PSUM accumulation group per block bank: the first self matmul
    # starts it (start=True zeroes the whole 2KB bank), the absolute last
    # aggregation matmul of the block stops it.
    last_mm_of_blk = {}
    for cl in calls:
        if cl["mms"]:
            last_mm_of_blk[cl["blk"]] = cl["mms"][-1]
    stopset = set(last_mm_of_blk.values())
    assert len(stopset) == nblk, "every block must have edge matmuls"
    for cl in calls:
        cl["flags"] = [(m, m in stopset) for m in cl["mms"]]

    ncalls = len(calls)
    meta_t = [np.zeros((128, MC * ncalls), np.int16) for _ in range(nc_)]
    drow_t = [np.full((1, NI * ncalls), 300.0, BF) for _ in range(nc_)]
    for c in range(nc_):
        for i in range(ncalls):
            meta_t[c][:, i * MC:(i + 1) * MC] = pc_meta[c][i]
            ni = calls[i]["ni"]
            drow_t[c][0, i * NI:i * NI + ni] = pc_drow[c][i]

    st = dict(calls=calls, nwin=nwin, nblk=nblk, ncalls=ncalls, MC=MC)
    percore = [dict(meta=meta_t[c], drow=drow_t[c]) for c in range(nc_)]
    return st, percore


# ---------------------------------------------------------------- program
def build_nc(cfg, st):
    import concourse.bass as bass
    import concourse.bacc as bacc
    import concourse.tile as tile
    import concourse.mybir as mybir
    from concourse.masks import make_identity

    bf16, f32 = mybir.dt.bfloat16, mybir.dt.float32
    i16, i32 = mybir.dt.int16, mybir.dt.int32
    AL = mybir.AluOpType
    AF = mybir.ActivationFunctionType
    ax_x = mybir.AxisListType.X

    nc_, nsh, npad, wb = cfg["ncores"], cfg["nshard"], cfg["npad"], cfg["wb"]
    ngsh, NI = cfg["ngsh"], cfg["ni_max"]
    H, C1, CL = cfg["heads"], cfg["hid"], cfg["classes"]
    D1 = H * C1                      # 64
    NEGS = cfg["neg"]
    nwin, nblk, ncalls = st["nwin"], st["nblk"], st["ncalls"]
    MC = st["MC"]
    NTOT = nc_ * npad
    GS = NTOT // ngsh                # 25088
    ntile_x = npad // 128            # 98
    RH1 = D1 + H                     # 72
    RH2 = D1 + 1                     # 65
    W1C = D1 + 2 * H                 # 80

    nc = bacc.Bacc("TRN2", target_bir_lowering=False, debug=False,
                   enable_asserts=False, num_devices=nc_, num_swdge_queues=4)

    # ---- I/O
    x_T = nc.dram_tensor("x_T", [cfg["f_in"], npad], f32, kind="ExternalInput")
    w1cat = nc.dram_tensor("w1cat", [cfg["f_in"], W1C], f32,
                           kind="ExternalInput")
    b1row = nc.dram_tensor("b1row", [1, D1], f32, kind="ExternalInput")
    wa2s = nc.dram_tensor("wa2s", [1, D1], f32, kind="ExternalInput")
    wa2d = nc.dram_tensor("wa2d", [1, D1], f32, kind="ExternalInput")
    w2b = nc.dram_tensor("w2b", [D1, CL], bf16, kind="ExternalInput")
    b2row = nc.dram_tensor("b2row", [1, CL], f32, kind="ExternalInput")
    meta_in = nc.dram_tensor("meta_in", [128, MC * ncalls], i16,
                             kind="ExternalInput")
    drow_in = nc.dram_tensor("drow_in", [1, NI * ncalls], bf16,
                             kind="ExternalInput")
    out_d = nc.dram_tensor("out", [npad, CL], f32, kind="ExternalOutput")

    with tile.TileContext(nc) as tc:
        with (
            tc.tile_pool(name="const", bufs=1) as cpool,
            tc.tile_pool(name="res", bufs=1) as rp,
            tc.tile_pool(name="p0", bufs=3) as p0,
            tc.tile_pool(name="meta", bufs=6) as mp,
            tc.tile_pool(name="drp", bufs=3) as dpp,
            tc.tile_pool(name="gpool", bufs=4) as gp,
            tc.tile_pool(name="spool", bufs=2) as sp,
            tc.tile_pool(name="rhsp", bufs=3) as rhp,
            tc.tile_pool(name="tv", bufs=3) as tvp,
            tc.tile_pool(name="epi", bufs=2) as ep,
            tc.tile_pool(name="pwin", bufs=2, space="PSUM") as pw,
            tc.tile_pool(name="pald", bufs=2, space="PSUM") as pa,
            tc.tile_pool(name="pm", bufs=2, space="PSUM") as pm,
            tc.tile_pool(name="ptr", bufs=1, space="PSUM") as ptp,
            tc.tile_pool(name="dram", bufs=1, space="DRAM") as dp,
        ):
            # ---------- constants
            ident = cpool.tile([128, 128], f32)
            make_identity(nc, ident[:])
            identb = cpool.tile([128, 128], bf16)
            nc.vector.tensor_copy(identb[:], ident[:])
            iota_i = cpool.tile([128, 128], i32)
            nc.gpsimd.iota(iota_i[:], pattern=[[1, 128]], base=0,
                           channel_multiplier=0)
            iota_mat = cpool.tile([128, 128], bf16)
            nc.vector.tensor_copy(iota_mat[:], iota_i[:])
            iota_mat2 = cpool.tile([128, 128], bf16)
            nc.vector.tensor_scalar_add(iota_mat2[:], iota_mat[:], 128.0)
            ic_i = cpool.tile([128, 1], i32)
            nc.gpsimd.iota(ic_i[:], pattern=[[0, 1]], base=0,
                           channel_multiplier=1)
            iota_col = cpool.tile([128, 1], bf16)
            nc.vector.tensor_copy(iota_col[:], ic_i[:])
            iotacol_w = cpool.tile([128, NI], bf16)
            nc.vector.tensor_copy(iotacol_w[:],
                                  iota_col[:].broadcast_to([128, NI]))
            iotacol2 = cpool.tile([128, 128], bf16)
            nc.vector.tensor_copy(
                iotacol2[:],
                iota_col[:].broadcast_to([128, 128]))
            nc.vector.tensor_scalar_add(iotacol2[:], iotacol2[:], 128.0)
            b1m = cpool.tile([128, D1], f32)
            nc.sync.dma_start(out=b1m[:], in_=b1row[:].to_broadcast([128, D1]))
            wa2sm = cpool.tile([128, D1], f32)
            nc.sync.dma_start(out=wa2sm[:], in_=wa2s[:].to_broadcast([128, D1]))
            wa2dm = cpool.tile([128, D1], f32)
            nc.sync.dma_start(out=wa2dm[:], in_=wa2d[:].to_broadcast([128, D1]))
            b2m = cpool.tile([128, CL], f32)
            nc.sync.dma_start(out=b2m[:], in_=b2row[:].to_broadcast([128, CL]))
            w1c_sb = cpool.tile([cfg["f_in"], W1C], f32)
            nc.sync.dma_start(out=w1c_sb[:], in_=w1cat[:])
            w2b_sb = cpool.tile([D1, CL], bf16)
            nc.sync.dma_start(out=w2b_sb[:], in_=w2b[:])

            # resident: Ed/ed window tables (hi/lo bf16) + self-loop rhs
            al1w = rp.tile([128, 32 * nwin], bf16)  # [Edhi8|edhi8|Edlo8|edlo8]
            al2w = rp.tile([128, 4 * nwin], bf16)   # [Edhi|edhi|Edlo|edlo]
            self1 = rp.tile([128, RH1 * nwin], bf16)
            self2 = rp.tile([128, RH2 * nwin], bf16)

            # DRAM tiles
            t1_own = dp.tile([npad, 128], bf16)
            t1_full = dp.tile([NTOT, 128], bf16)
            t2_own = dp.tile([npad, 128], bf16)
            t2_full = dp.tile([NTOT, 128], bf16)

            # ---------------- P0: produce T1 + al1/self1 tables
            for t in range(ntile_x):
                xt = p0.tile([cfg["f_in"], 128], f32, tag="xt")
                nc.sync.dma_start(out=xt[:], in_=x_T[:, t * 128:(t + 1) * 128])
                ps = pm.tile([128, W1C], f32, space="PSUM", tag="pm")
                nc.tensor.matmul(ps[:], lhsT=xt[:], rhs=w1c_sb[:],
                                 start=True, stop=True)
                t1sb = p0.tile([128, 128], bf16, tag="t1sb")
                nc.vector.tensor_copy(t1sb[:, 0:D1], ps[:, 0:D1])
                nc.scalar.activation(t1sb[:, D1:D1 + H], ps[:, D1:D1 + H],
                                     AF.Exp)
                nc.scalar.activation(t1sb[:, D1 + H:D1 + 2 * H],
                                     ps[:, D1:D1 + H], AF.Exp, scale=NEGS)
                nc.vector.memset(t1sb[:, D1 + 2 * H:128], 0.0)
                Edf = p0.tile([128, H], f32, tag="Edf")
                nc.scalar.activation(Edf[:], ps[:, D1 + H:W1C], AF.Exp)
                edf = p0.tile([128, H], f32, tag="edf")
                nc.scalar.activation(edf[:], ps[:, D1 + H:W1C], AF.Exp,
                                     scale=NEGS)
                o = 32 * t
                nc.vector.tensor_copy(al1w[:, o:o + H], Edf[:])
                nc.vector.tensor_copy(al1w[:, o + H:o + 2 * H], edf[:])
                nc.vector.tensor_tensor(out=al1w[:, o + 16:o + 24],
                                        in0=Edf[:], in1=al1w[:, o:o + H],
                                        op=AL.subtract)
                nc.vector.tensor_tensor(out=al1w[:, o + 24:o + 32],
                                        in0=edf[:], in1=al1w[:, o + H:o + 16],
                                        op=AL.subtract)
                # self-loop alpha + rhs
                av = p0.tile([128, H], f32, tag="av")
                nc.vector.tensor_tensor(out=av[:], in0=t1sb[:, D1:D1 + H],
                                        in1=Edf[:], op=AL.mult)
                bv = p0.tile([128, H], f32, tag="bv")
                nc.vector.tensor_tensor(out=bv[:],
                                        in0=t1sb[:, D1 + H:D1 + 2 * H],
                                        in1=edf[:], op=AL.mult)
                so = RH1 * t
                nc.vector.tensor_tensor(out=self1[:, so + D1:so + RH1],
                                        in0=av[:], in1=bv[:], op=AL.max)
                nc.vector.tensor_tensor(
                    out=self1[:, so:so + D1].rearrange("p (a c) -> p a c",
                                                       c=C1),
                    in0=t1sb[:, 0:D1].rearrange("p (a c) -> p a c", c=C1),
                    in1=self1[:, so + D1:so + RH1]
                    .broadcast_to([128, H, C1]),
                    op=AL.mult)
                nc.sync.dma_start(out=t1_own[t * 128:(t + 1) * 128, :],
                                  in_=t1sb[:])

            nc.gpsimd.collective_compute(
                "AllGather", AL.bypass,
                replica_groups=[list(range(nc_))],
                ins=[t1_own.opt()], outs=[t1_full.opt()],
            )

            # ---------------- shared edge pass
            def edge_pass(tfull, alw, selfw, nal, rhw, aws):
                """nal: attention scalars/edge; rhw: rhs width; aws: alw stride."""
                call_i = 0
                calls_by_blk = [[] for _ in range(nblk)]
                for ci, cl in enumerate(st["calls"]):
                    calls_by_blk[cl["blk"]].append(ci)
                for b in range(nblk):
                    ptb = pw.tile([128, wb * RH1], f32, space="PSUM",
                                  tag="pwin")
                    w0 = b * wb
                    for wi in range(wb):
                        w = w0 + wi
                        nc.tensor.matmul(
                            ptb[:, wi * RH1:wi * RH1 + rhw], lhsT=identb[:],
                            rhs=selfw[:, w * rhw:(w + 1) * rhw],
                            start=(wi == 0), stop=False)
                    for ci in calls_by_blk[b]:
                        cl = st["calls"][ci]
                        ni, nt = cl["ni"], cl["ntile"]
                        meta = mp.tile([128, MC], i16, tag="meta")
                        nc.sync.dma_start(out=meta[:],
                                          in_=meta_in[:, ci * MC:(ci + 1) * MC])
                        dcol = meta[:, NI // 16:NI // 16 + nt].bitcast(bf16)
                        drep = dpp.tile([128, NI], bf16, tag="drep")
                        nc.sync.dma_start(
                            out=drep[:, 0:ni],
                            in_=drow_in[:, ci * NI:ci * NI + ni]
                            .to_broadcast([128, ni]))
                        g = gp.tile([128, NI], bf16, tag="g")
                        s_ = cl["shard"]
                        nc.gpsimd.dma_gather(
                            g[:, 0:ni].rearrange("p (b e) -> p b e", e=128),
                            tfull[s_ * GS:(s_ + 1) * GS, :],
                            meta[:, 0:ni // 16], ni, ni, 128,
                            single_packet=False, queue_num=call_i % 4)
                        call_i += 1
                        # one-hot builds
                        s1 = sp.tile([128, NI], bf16, tag="s1")
                        im1 = iota_mat[:].rearrange("p (o n) -> p o n", o=1)
                        nc.vector.tensor_tensor(
                            out=s1[:, 0:ni].rearrange("p (b n) -> p b n",
                                                      n=128),
                            in0=dcol.broadcast_to([128, nt, 128]),
                            in1=im1.broadcast_to([128, nt, 128]),
                            op=AL.is_equal)
                        st1 = sp.tile([128, NI], bf16, tag="st1")
                        nc.vector.tensor_tensor(
                            out=st1[:, 0:ni], in0=iotacol_w[:, 0:ni],
                            in1=drep[:, 0:ni], op=AL.is_equal)
                        strads = [t for t, (w1, sdl) in enumerate(cl["tiles"])
                                  if sdl]
                        nstr = len(strads)
                        if nstr:
                            s2s = sp.tile([128, 128 * 8], bf16, tag="s2s")
                            st2s = sp.tile([128, 128 * 8], bf16, tag="st2s")
                            for j, t in enumerate(strads):
                                nc.vector.tensor_tensor(
                                    out=s2s[:, j * 128:(j + 1) * 128],
                                    in0=dcol[:, t:t + 1]
                                    .broadcast_to([128, 128]),
                                    in1=iota_mat2[:], op=AL.is_equal)
                                nc.vector.tensor_tensor(
                                    out=st2s[:, j * 128:(j + 1) * 128],
                                    in0=iotacol2[:],
                                    in1=drep[:, t * 128:(t + 1) * 128],
                                    op=AL.is_equal)
                        sidx = {t: j for j, t in enumerate(strads)}
                        # Ed/ed lookup (hi/lo accumulated in f32 psum)
                        pald = pa.tile([128, 62 * 8], f32, space="PSUM",
                                       tag="pald")
                        na2 = 2 * nal
                        for t, (w1, sdl) in enumerate(cl["tiles"]):
                            po = pald[:, t * na2:(t + 1) * na2]
                            last_t = t == nt - 1
                            nc.tensor.matmul(
                                po, lhsT=st1[:, t * 128:(t + 1) * 128],
                                rhs=alw[:, aws * w1:aws * w1 + na2],
                                start=(t == 0), stop=False)
                            nc.tensor.matmul(
                                po, lhsT=st1[:, t * 128:(t + 1) * 128],
                                rhs=alw[:, aws * w1 + na2:aws * (w1 + 1)],
                                start=False, stop=last_t and not sdl)
                            if sdl:
                                j = sidx[t]
                                sl = st2s[:, j * 128:(j + 1) * 128]
                                nc.tensor.matmul(
                                    po, lhsT=sl,
                                    rhs=alw[:, aws * (w1 + 1):
                                            aws * (w1 + 1) + na2],
                                    start=False, stop=False)
                                nc.tensor.matmul(
                                    po, lhsT=sl,
                                    rhs=alw[:, aws * (w1 + 1) + na2:
                                            aws * (w1 + 2)],
                                    start=False, stop=last_t)
                        # alpha = max(Es*Ed, es*ed); rhs = [msg*alpha | alpha]
                        gv = g[:, 0:ni].rearrange("p (b e) -> p b e", e=128)
                        pv = pald[:, 0:nt * na2].rearrange(
                            "p (b a) -> p b a", a=na2)
                        t1v = tvp.tile([128, 8 * 32], f32, tag="t1v")
                        t2v = tvp.tile([128, 8 * 32], f32, tag="t2v")
                        t1vv = t1v[:, 0:nt * nal].rearrange(
                            "p (b a) -> p b a", a=nal)
                        t2vv = t2v[:, 0:nt * nal].rearrange(
                            "p (b a) -> p b a", a=nal)
                        nc.vector.tensor_tensor(
                            out=t1vv, in0=gv[:, :, D1:D1 + nal],
                            in1=pv[:, :, 0:nal], op=AL.mult)
                        nc.vector.tensor_tensor(
                            out=t2vv, in0=gv[:, :, D1 + nal:D1 + 2 * nal],
                            in1=pv[:, :, nal:na2], op=AL.mult)
                        rhs = rhp.tile([128, RH1 * 32], bf16, tag="rhs")
                        rv = rhs[:, 0:nt * rhw].rearrange(
                            "p (b r) -> p b r", r=rhw)
                        nc.vector.tensor_tensor(
                            out=rv[:, :, D1:D1 + nal], in0=t1vv, in1=t2vv,
                            op=AL.max)
                        cph = D1 // nal
                        nc.vector.tensor_tensor(
                            out=rv[:, :, 0:D1].rearrange(
                                "p b (a c) -> p b a c", c=cph),
                            in0=gv[:, :, 0:D1].rearrange(
                                "p b (a c) -> p b a c", c=cph),
                            in1=rv[:, :, D1:D1 + nal]
                            .broadcast_to([128, nt, nal, cph]),
                            op=AL.mult)
                        # aggregation matmuls
                        for (mm, fstop) in cl["flags"]:
                            _, t, k, w = mm
                            wi = w - w0
                            if k == 0:
                                lhsT = s1[:, t * 128:(t + 1) * 128]
                            else:
                                j = sidx[t]
                                lhsT = s2s[:, j * 128:(j + 1) * 128]
                            nc.tensor.matmul(
                                ptb[:, wi * RH1:wi * RH1 + rhw], lhsT=lhsT,
                                rhs=rhs[:, t * rhw:(t + 1) * rhw],
                                start=False, stop=fstop)
                    yield b, ptb

            # ---------------- L1 pass + epilogue -> T2
            for b, ptb in edge_pass(t1_full, al1w, self1, H, RH1, 32):
                w0 = b * wb
                ptv = ptb[:].rearrange("p (w r) -> p w r", r=RH1)
                rc = ep.tile([128, wb * H], f32, tag="rc1")
                rcv = rc[:].rearrange("p (w a) -> p w a", a=H)
                nc.vector.reciprocal(rcv, ptv[:, :, D1:RH1])
                nc.vector.tensor_scalar_min(rc[:], rc[:], 1e30)
                o1 = ep.tile([128, wb * D1], f32, tag="o1")
                o1v = o1[:].rearrange("p (w a c) -> p w a c", a=H, c=C1)
                nc.vector.tensor_tensor(
                    out=o1v,
                    in0=ptv[:, :, 0:D1].rearrange("p w (a c) -> p w a c",
                                                  c=C1),
                    in1=rcv.broadcast_to([128, wb, H, C1]), op=AL.mult)
                b1v = b1m[:].rearrange("p (o c) -> p o c", o=1)
                o1w = o1[:].rearrange("p (w c) -> p w c", c=D1)
                nc.vector.tensor_tensor(
                    out=o1w, in0=o1w,
                    in1=b1v.broadcast_to([128, wb, D1]), op=AL.add)
                ra = ep.tile([128, wb * D1], f32, tag="ra")
                nc.scalar.activation(ra[:], o1[:], AF.Relu)
                rav = ra[:].rearrange("p (w c) -> p w c", c=D1)
                t2sb = ep.tile([128, wb * 128], bf16, tag="t2sb")
                t2v_ = t2sb[:].rearrange("p (w c) -> p w c", c=128)
                nc.vector.tensor_copy(t2v_[:, :, 0:D1], rav)
                nc.vector.memset(t2v_[:, :, D1 + 2:128], 0.0)
                tmp = ep.tile([128, wb * D1], f32, tag="altmp")
                tmpv = tmp[:].rearrange("p (w c) -> p w c", c=D1)
                a2 = ep.tile([128, 2 * wb], f32, tag="a2")
                wsv = wa2sm[:].rearrange("p (o c) -> p o c", o=1)
                nc.vector.tensor_tensor(out=tmpv, in0=rav,
                                        in1=wsv.broadcast_to([128, wb, D1]),
                                        op=AL.mult)
                nc.vector.tensor_reduce(
                    a2[:, 0:wb].rearrange("p (w o) -> p w o", o=1), tmpv,
                    axis=ax_x, op=AL.add)
                wdv = wa2dm[:].rearrange("p (o c) -> p o c", o=1)
                nc.vector.tensor_tensor(out=tmpv, in0=rav,
                                        in1=wdv.broadcast_to([128, wb, D1]),
                                        op=AL.mult)
                nc.vector.tensor_reduce(
                    a2[:, wb:2 * wb].rearrange("p (w o) -> p w o", o=1), tmpv,
                    axis=ax_x, op=AL.add)
                # Es2/es2 into table; Ed2/ed2 hi/lo into al2w; self2
                nc.scalar.activation(t2v_[:, :, D1:D1 + 1], a2[:, 0:wb]
                                     .rearrange("p (w o) -> p w o", o=1), AF.Exp)
                nc.scalar.activation(t2v_[:, :, D1 + 1:D1 + 2], a2[:, 0:wb]
                                     .rearrange("p (w o) -> p w o", o=1), AF.Exp,
                                     scale=NEGS)
                Ed2 = ep.tile([128, 2 * wb], f32, tag="Ed2")
                nc.scalar.activation(Ed2[:, 0:wb], a2[:, wb:2 * wb], AF.Exp)
                nc.scalar.activation(Ed2[:, wb:2 * wb], a2[:, wb:2 * wb],
                                     AF.Exp, scale=NEGS)
                awv = al2w[:, 4 * w0:4 * (w0 + wb)].rearrange(
                    "p (w k) -> p w k", k=4)
                E2v = Ed2[:, 0:wb].rearrange("p (w o) -> p w o", o=1)
                e2v = Ed2[:, wb:2 * wb].rearrange("p (w o) -> p w o", o=1)
                nc.vector.tensor_copy(awv[:, :, 0:1], E2v)
                nc.vector.tensor_copy(awv[:, :, 1:2], e2v)
                nc.vector.tensor_tensor(out=awv[:, :, 2:3], in0=E2v,
                                        in1=awv[:, :, 0:1], op=AL.subtract)
                nc.vector.tensor_tensor(out=awv[:, :, 3:4], in0=e2v,
                                        in1=awv[:, :, 1:2], op=AL.subtract)
                sa = ep.tile([128, 2 * wb], f32, tag="sa")
                nc.vector.tensor_tensor(
                    out=sa[:, 0:wb].rearrange("p (w o) -> p w o", o=1),
                    in0=t2v_[:, :, D1:D1 + 1], in1=E2v, op=AL.mult)
                nc.vector.tensor_tensor(
                    out=sa[:, wb:2 * wb].rearrange("p (w o) -> p w o", o=1),
                    in0=t2v_[:, :, D1 + 1:D1 + 2], in1=e2v, op=AL.mult)
                s2v = self2[:, RH2 * w0:RH2 * (w0 + wb)].rearrange(
                    "p (w r) -> p w r", r=RH2)
                nc.vector.tensor_tensor(
                    out=s2v[:, :, D1:RH2],
                    in0=sa[:, 0:wb].rearrange("p (w o) -> p w o", o=1),
                    in1=sa[:, wb:2 * wb].rearrange("p (w o) -> p w o", o=1),
                    op=AL.max)
                nc.vector.tensor_tensor(
                    out=s2v[:, :, 0:D1], in0=t2v_[:, :, 0:D1],
                    in1=s2v[:, :, D1:RH2].broadcast_to([128, wb, D1]),
                    op=AL.mult)
                nc.sync.dma_start(
                    out=t2_own[w0 * 128:(w0 + wb) * 128, :]
                    .rearrange("(w p) c -> p w c", p=128),
                    in_=t2v_)

            nc.gpsimd.collective_compute(
                "AllGather", AL.bypass,
                replica_groups=[list(range(nc_))],
                ins=[t2_own.opt()], outs=[t2_full.opt()],
            )

            # ---------------- L2 pass + epilogue -> output
            for b, ptb in edge_pass(t2_full, al2w, self2, 1, RH2, 4):
                w0 = b * wb
                ptv = ptb[:].rearrange("p (w r) -> p w r", r=RH1)
                rc = ep.tile([128, wb], f32, tag="rc2")
                nc.vector.reciprocal(rc[:].rearrange("p (w o) -> p w o", o=1),
                                     ptv[:, :, D1:D1 + 1])
                nc.vector.tensor_scalar_min(rc[:], rc[:], 1e30)
                tca = ep.tile([128, wb * D1], bf16, tag="tca")
                tcav = tca[:].rearrange("p (w c) -> p w c", c=D1)
                nc.vector.tensor_copy(tcav, ptv[:, :, 0:D1])
                lg = ep.tile([128, wb * CL], f32, tag="lg")
                for wi in range(wb):
                    trp = ptp.tile([D1, 128], bf16, space="PSUM", tag="trp")
                    nc.tensor.transpose(out=trp[:],
                                        in_=tca[:, wi * D1:(wi + 1) * D1],
                                        identity=identb[:])
                    trs = ep.tile([D1, 128], bf16, tag="trs")
                    nc.vector.tensor_copy(trs[:], trp[:])
                    op2 = ptp.tile([128, CL], f32, space="PSUM", tag="op2")
                    nc.tensor.matmul(op2[:], lhsT=trs[:], rhs=w2b_sb[:],
                                     start=True, stop=True)
                    nc.vector.scalar_tensor_tensor(
                        out=lg[:, wi * CL:(wi + 1) * CL], in0=op2[:],
                        scalar=rc[:, wi:wi + 1], in1=b2m[:],
                        op0=AL.mult, op1=AL.add)
                lgv = lg[:].rearrange("p (w c) -> p w c", c=CL)
                mx = ep.tile([128, wb], f32, tag="mx")
                mxv = mx[:].rearrange("p (w o) -> p w o", o=1)
                nc.vector.tensor_reduce(mxv, lgv, axis=ax_x, op=AL.max)
                nc.vector.tensor_tensor(out=lgv, in0=lgv,
                                        in1=mxv.broadcast_to([128, wb, CL]),
                                        op=AL.subtract)
                exs = ep.tile([128, wb * CL], f32, tag="exs")
                nc.scalar.activation(exs[:], lg[:], AF.Exp)
                sm = ep.tile([128, wb], f32, tag="sm")
                smv = sm[:].rearrange("p (w o) -> p w o", o=1)
                nc.vector.tensor_reduce(
                    smv, exs[:].rearrange("p (w c) -> p w c", c=CL),
                    axis=ax_x, op=AL.add)
                lnm = ep.tile([128, wb], f32, tag="lnm")
                nc.scalar.activation(lnm[:], sm[:], AF.Ln)
                nc.vector.tensor_tensor(
                    out=lgv, in0=lgv,
                    in1=lnm[:].rearrange("p (w o) -> p w o", o=1)
                    .broadcast_to([128, wb, CL]),
                    op=AL.subtract)
                nc.sync.dma_start(
                    out=out_d[w0 * 128:(w0 + wb) * 128, :]
                    .rearrange("(w p) c -> p w c", p=128),
                    in_=lgv)

    nc.compile()
    return nc


def _host_inputs(inputs, cfg, percore):
    x = np.asarray(inputs["x"], np.float32)
    W1 = np.asarray(inputs["W1"], np.float32)
    a_s1 = np.asarray(inputs["a_src1"], np.float32)
    a_d1 = np.asarray(inputs["a_dst1"], np.float32)
    b1 = np.asarray(inputs["b1"], np.float32)
    W2 = np.asarray(inputs["W2"], np.float32)
    a_s2 = np.asarray(inputs["a_src2"], np.float32)
    a_d2 = np.asarray(inputs["a_dst2"], np.float32)
    b2 = np.asarray(inputs["b2"], np.float32)
    H, C1 = cfg["heads"], cfg["hid"]
    D1 = H * C1
    As = np.zeros((D1, H), np.float32)
    Ad = np.zeros((D1, H), np.float32)
    for hd in range(H):
        As[hd * C1:(hd + 1) * C1, hd] = a_s1[hd]
        Ad[hd * C1:(hd + 1) * C1, hd] = a_d1[hd]
    w1cat = np.concatenate([W1, W1 @ As, W1 @ Ad], axis=1)
    wa2s = (W2 @ a_s2[0])[None, :]
    wa2d = (W2 @ a_d2[0])[None, :]
    nsh, npad = cfg["nshard"], cfg["npad"]
    maps = []
    for c in range(cfg["ncores"]):
        xs = x[c * nsh:(c + 1) * nsh]
        xp = np.zeros((npad, cfg["f_in"]), np.float32)
        xp[:xs.shape[0]] = xs
        maps.append(dict(
            x_T=np.ascontiguousarray(xp.T), w1cat=w1cat,
            b1row=b1[None, :], wa2s=wa2s, wa2d=wa2d,
            w2b=W2.astype(BF), b2row=b2[None, :],
            meta_in=percore[c]["meta"], drow_in=percore[c]["drow"],
        ))
    return maps


_CACHE = {}


def kernel(**inputs):
    from concourse import bass_utils

    cfg = FULL_CFG
    ei = np.asarray(inputs["edge_index"])
    src = ei[0].astype(np.int64)
    dst = ei[1].astype(np.int64)

    key = ("full", ei.shape[1])
    if key not in _CACHE:
        st, percore = prep_structure(src, dst, cfg)
        ncobj = build_nc(cfg, st)
        _CACHE[key] = (st, percore, ncobj)
    st, percore, ncobj = _CACHE[key]

    in_maps = _host_inputs(inputs, cfg, percore)
    res = bass_utils.run_bass_kernel_spmd(
        ncobj, in_maps, core_ids=list(range(cfg["ncores"])))
    outs = [res.results[c]["out"][:cfg["nshard"]]
            for c in range(cfg["ncores"])]
    return np.concatenate(outs, axis=0).astype(np.float32)


# revision 23
# speedup vs baseline: 1.0462x; 1.0462x over previous
"""GAT (2-layer, 8-head then 1-head) on 8 Trainium2 NeuronCores.

v2 design: dst-shard nodes across 8 cores (12544-padded shards). Per layer,
each core builds a bf16 node table [npad, 128] = [msg(64) | Es | es | 0...]
where Es = exp(al_src), es = exp(0.2*al_src); AllGathered to all cores.
Attention uses the factorization
    exp(leaky(als+ald)) = max(Es*Ed, es*ed),   Ed = exp(al_dst), ed = exp(.2*al_dst)
so the per-edge work is two multiplies and a max; Ed/ed come from a per-window
resident table looked up on the PE via one-hot S^T matmuls (bf16 hi/lo pairs
accumulated in f32 PSUM for full precision).

Edges are dst-owned, grouped by (block of 7 windows, src-super-shard of 25088
rows, window) with core-common structure; h[src] rows are fetched with ONE
dma_gather per (block, super-shard) stream (~3.9k edges) to amortize the ~5us
SWDGE fixed cost. One-hot S matrices are built on DVE (st1 hits the 2x 16-bit
path against a constant iotacol_wide). Aggregation matmuls accumulate
[msg*alpha | alpha] into per-window PSUM slices packed 7-to-a-bank.
Self-loops are folded in as per-window SBUF-resident rhs computed during the
producer phase and added via one identity matmul per window. Layer 2
aggregates relu1 and applies W2 after aggregation; 1/denominator is applied
after the W2 matmul (row scalars commute).
"""
import sys
import numpy as np

sys.path.insert(0, "/opt/trn_rl_repo")
import ml_dtypes

BF = ml_dtypes.bfloat16

N = 100000
F_IN = 128
HID = 8
HEADS = 8
CLASSES = 40
NEG = 0.2
NC = 8

FULL_CFG = dict(
    ncores=8, nshard=12500, npad=12544, wb=7, ngsh=4, ni_max=3968,
    f_in=128, heads=8, hid=8, classes=40, neg=0.2,
)


def _ceil(a, b):
    return -(-a // b)


# ---------------------------------------------------------------- host prep
def prep_structure(src, dst, cfg):
    """Build core-common call/tile/matmul structure + per-core arrays.

    Returns (st, percore): st holds the shared program structure, percore[c]
    holds the packed meta (idx|dcol) and drow arrays for core c.
    """
    nc_, nsh, npad, wb = cfg["ncores"], cfg["nshard"], cfg["npad"], cfg["wb"]
    ngsh, NI = cfg["ngsh"], cfg["ni_max"]
    nwin = npad // 128                      # 98
    nblk = _ceil(nwin, wb)                  # 14
    assert nwin % wb == 0, "code assumes full blocks"
    gsz = (nc_ * npad) // ngsh              # 25088 rows per gather super-shard
    padrow = nsh                            # zero-ish row within super-shard

    core = dst // nsh
    dstl = dst % nsh
    g_row = (src // nsh) * npad + (src % nsh)
    gsh = g_row // gsz
    gloc = g_row % gsz
    win = dstl // 128
    blk = win // wb

    counts = np.zeros((nc_, nblk, ngsh, nwin), dtype=np.int64)
    np.add.at(counts, (core, blk, gsh, win), 1)
    common = counts.max(axis=0)             # [nblk, ngsh, nwin]
    for b in range(nblk):
        assert common[b, :, b * wb:(b + 1) * wb].min() >= 128, \
            "tile could span >2 windows"

    # per-core edge arrays sorted by (block, gshard, window)
    percore_edges = []
    for c in range(nc_):
        m = core == c
        key = (blk[m].astype(np.int64) * ngsh + gsh[m]) * nwin + win[m]
        o = np.argsort(key, kind="stable")
        percore_edges.append((key[o], gloc[m][o], dstl[m][o], win[m][o]))

    calls = []
    pc_meta = [[] for _ in range(nc_)]
    pc_drow = [[] for _ in range(nc_)]
    win_mms = {}                            # (b, w) -> list of mm ids
    MC = NI // 16 + 32                      # meta cols (idx | dcol-as-i16)

    for b in range(nblk):
        for s in range(ngsh):
            streams = []
            for c in range(nc_):
                kk, sl, dl, wn = percore_edges[c]
                segs = []
                for w in range(b * wb, (b + 1) * wb):
                    kval = (b * ngsh + s) * nwin + w
                    lo = np.searchsorted(kk, kval, "left")
                    hi = np.searchsorted(kk, kval, "right")
                    n_common = common[b, s, w]
                    seg_s = np.full(n_common, padrow, dtype=np.int64)
                    seg_w = np.full(n_common, w, dtype=np.int64)
                    seg_d = np.full(n_common, -1, dtype=np.int64)
                    seg_s[: hi - lo] = sl[lo:hi]
                    seg_w[: hi - lo] = wn[lo:hi]
                    seg_d[: hi - lo] = dl[lo:hi]
                    segs.append(np.stack([seg_s, seg_w, seg_d]))
                streams.append(np.concatenate(segs, axis=1))
            L = streams[0].shape[1]
            pos = 0
            while pos < L:
                ni_real = min(NI, L - pos)
                ni = _ceil(ni_real, 128) * 128
                ntile = ni // 128
                wseg = streams[0][1][pos:pos + ni_real]
                tiles = []
                for t in range(ntile):
                    a, z = t * 128, min((t + 1) * 128, ni_real)
                    if a < ni_real:
                        tw = wseg[a:z]
                        w1 = int(tw.min())
                        assert int(tw.max()) - w1 <= 1, "tile spans >2 windows"
                        straddle = int(tw.max()) > w1
                    else:
                        w1, straddle = int(wseg[-1]), False
                    tiles.append((w1, straddle))
                cid = len(calls)
                mms = []
                for t, (w1, straddle) in enumerate(tiles):
                    for k in ([0, 1] if straddle else [0]):
                        mm_id = (cid, t, k, w1 + k)
                        win_mms.setdefault((b, w1 + k), []).append(mm_id)
                        mms.append(mm_id)
                calls.append(dict(blk=b, shard=s, ni=ni, ntile=ntile,
                                  tiles=tiles, mms=mms))
                for c in range(nc_):
                    ss, ww, dd = streams[c]
                    sl_call = np.full(ni, padrow, dtype=np.int64)
                    rel_call = np.full(ni, 300.0, dtype=np.float64)
                    nreal = min(ni_real, L - pos)
                    sl_call[:nreal] = ss[pos:pos + nreal]
                    for t in range(ntile):
                        a, z = t * 128, min((t + 1) * 128, nreal)
                        if a >= nreal:
                            break
                        w1 = tiles[t][0]
                        dv = dd[pos + a:pos + z]
                        wv = ww[pos + a:pos + z]
                        rel = (wv - w1) * 128 + (dv - wv * 128)
                        rel = np.where(dv < 0, 300.0, rel)
                        rel_call[a:z] = rel
                    # meta: idx wrapped [16, ni/16] tiled x8, then dcol bf16 bits
                    meta = np.zeros((128, MC), np.int16)
                    iw = sl_call.reshape(ni // 16, 16).T.astype(np.int16)
                    meta[:, 0:ni // 16] = np.tile(iw, (8, 1))
                    dcol = rel_call.reshape(ntile, 128).T.astype(BF)
                    meta[:, NI // 16:NI // 16 + ntile] = dcol.view(np.int16)
                    pc_meta[c].append(meta)
                    pc_drow[c].append(rel_call.astype(BF))
                pos += ni_real

    # One PSUM accumulation group per block bank: the first self matmul
    # starts it (start=True zeroes the whole 2KB bank), the absolute last
    # aggregation matmul of the block stops it.
    last_mm_of_blk = {}
    for cl in calls:
        if cl["mms"]:
            last_mm_of_blk[cl["blk"]] = cl["mms"][-1]
    stopset = set(last_mm_of_blk.values())
    assert len(stopset) == nblk, "every block must have edge matmuls"
    for cl in calls:
        cl["flags"] = [(m, m in stopset) for m in cl["mms"]]

    ncalls = len(calls)
    meta_t = [np.zeros((128, MC * ncalls), np.int16) for _ in range(nc_)]
    drow_t = [np.full((1, NI * ncalls), 300.0, BF) for _ in range(nc_)]
    for c in range(nc_):
        for i in range(ncalls):
            meta_t[c][:, i * MC:(i + 1) * MC] = pc_meta[c][i]
            ni = calls[i]["ni"]
            drow_t[c][0, i * NI:i * NI + ni] = pc_drow[c][i]

    st = dict(calls=calls, nwin=nwin, nblk=nblk, ncalls=ncalls, MC=MC)
    percore = [dict(meta=meta_t[c], drow=drow_t[c]) for c in range(nc_)]
    return st, percore


# ---------------------------------------------------------------- program
def build_nc(cfg, st):
    import concourse.bass as bass
    import concourse.bacc as bacc
    import concourse.tile as tile
    import concourse.mybir as mybir
    from concourse.masks import make_identity

    bf16, f32 = mybir.dt.bfloat16, mybir.dt.float32
    i16, i32 = mybir.dt.int16, mybir.dt.int32
    AL = mybir.AluOpType
    AF = mybir.ActivationFunctionType
    ax_x = mybir.AxisListType.X

    nc_, nsh, npad, wb = cfg["ncores"], cfg["nshard"], cfg["npad"], cfg["wb"]
    ngsh, NI = cfg["ngsh"], cfg["ni_max"]
    H, C1, CL = cfg["heads"], cfg["hid"], cfg["classes"]
    D1 = H * C1                      # 64
    NEGS = cfg["neg"]
    nwin, nblk, ncalls = st["nwin"], st["nblk"], st["ncalls"]
    MC = st["MC"]
    NTOT = nc_ * npad
    GS = NTOT // ngsh                # 25088
    ntile_x = npad // 128            # 98
    RH1 = D1 + H                     # 72
    RH2 = D1 + 1                     # 65
    W1C = D1 + 2 * H                 # 80

    nc = bacc.Bacc("TRN2", target_bir_lowering=False, debug=False,
                   enable_asserts=False, num_devices=nc_, num_swdge_queues=4)

    # ---- I/O
    x_T = nc.dram_tensor("x_T", [cfg["f_in"], npad], f32, kind="ExternalInput")
    w1cat = nc.dram_tensor("w1cat", [cfg["f_in"], W1C], f32,
                           kind="ExternalInput")
    b1row = nc.dram_tensor("b1row", [1, D1], f32, kind="ExternalInput")
    wa2s = nc.dram_tensor("wa2s", [1, D1], f32, kind="ExternalInput")
    wa2d = nc.dram_tensor("wa2d", [1, D1], f32, kind="ExternalInput")
    w2b = nc.dram_tensor("w2b", [D1, CL], bf16, kind="ExternalInput")
    b2row = nc.dram_tensor("b2row", [1, CL], f32, kind="ExternalInput")
    meta_in = nc.dram_tensor("meta_in", [128, MC * ncalls], i16,
                             kind="ExternalInput")
    drow_in = nc.dram_tensor("drow_in", [1, NI * ncalls], bf16,
                             kind="ExternalInput")
    out_d = nc.dram_tensor("out", [npad, CL], f32, kind="ExternalOutput")

    with tile.TileContext(nc) as tc:
        with (
            tc.tile_pool(name="const", bufs=1) as cpool,
            tc.tile_pool(name="res", bufs=1) as rp,
            tc.tile_pool(name="p0", bufs=3) as p0,
            tc.tile_pool(name="meta", bufs=6) as mp,
            tc.tile_pool(name="drp", bufs=3) as dpp,
            tc.tile_pool(name="gpool", bufs=4) as gp,
            tc.tile_pool(name="spool", bufs=2) as sp,
            tc.tile_pool(name="rhsp", bufs=3) as rhp,
            tc.tile_pool(name="tv", bufs=3) as tvp,
            tc.tile_pool(name="epi", bufs=2) as ep,
            tc.tile_pool(name="pwin", bufs=2, space="PSUM") as pw,
            tc.tile_pool(name="pald", bufs=2, space="PSUM") as pa,
            tc.tile_pool(name="pm", bufs=2, space="PSUM") as pm,
            tc.tile_pool(name="ptr", bufs=1, space="PSUM") as ptp,
            tc.tile_pool(name="dram", bufs=1, space="DRAM") as dp,
        ):
            # ---------- constants
            ident = cpool.tile([128, 128], f32)
            make_identity(nc, ident[:])
            identb = cpool.tile([128, 128], bf16)
            nc.vector.tensor_copy(identb[:], ident[:])
            iota_i = cpool.tile([128, 128], i32)
            nc.gpsimd.iota(iota_i[:], pattern=[[1, 128]], base=0,
                           channel_multiplier=0)
            iota_mat = cpool.tile([128, 128], bf16)
            nc.vector.tensor_copy(iota_mat[:], iota_i[:])
            iota_mat2 = cpool.tile([128, 128], bf16)
            nc.vector.tensor_scalar_add(iota_mat2[:], iota_mat[:], 128.0)
            ic_i = cpool.tile([128, 1], i32)
            nc.gpsimd.iota(ic_i[:], pattern=[[0, 1]], base=0,
                           channel_multiplier=1)
            iota_col = cpool.tile([128, 1], bf16)
            nc.vector.tensor_copy(iota_col[:], ic_i[:])
            iotacol_w = cpool.tile([128, NI], bf16)
            nc.vector.tensor_copy(iotacol_w[:],
                                  iota_col[:].broadcast_to([128, NI]))
            iotacol2 = cpool.tile([128, 128], bf16)
            nc.vector.tensor_copy(
                iotacol2[:],
                iota_col[:].broadcast_to([128, 128]))
            nc.vector.tensor_scalar_add(iotacol2[:], iotacol2[:], 128.0)
            b1m = cpool.tile([128, D1], f32)
            nc.sync.dma_start(out=b1m[:], in_=b1row[:].to_broadcast([128, D1]))
            wa2sm = cpool.tile([128, D1], f32)
            nc.sync.dma_start(out=wa2sm[:], in_=wa2s[:].to_broadcast([128, D1]))
            wa2dm = cpool.tile([128, D1], f32)
            nc.sync.dma_start(out=wa2dm[:], in_=wa2d[:].to_broadcast([128, D1]))
            b2m = cpool.tile([128, CL], f32)
            nc.sync.dma_start(out=b2m[:], in_=b2row[:].to_broadcast([128, CL]))
            w1c_sb = cpool.tile([cfg["f_in"], W1C], f32)
            nc.sync.dma_start(out=w1c_sb[:], in_=w1cat[:])
            w2b_sb = cpool.tile([D1, CL], bf16)
            nc.sync.dma_start(out=w2b_sb[:], in_=w2b[:])
            clampc = cpool.tile([128, 1], f32)
            nc.vector.memset(clampc[:], 1e30)

            # resident: Ed/ed window tables (hi/lo bf16) + self-loop rhs
            al1w = rp.tile([128, 32 * nwin], bf16)  # [Edhi8|edhi8|Edlo8|edlo8]
            al2w = rp.tile([128, 4 * nwin], bf16)   # [Edhi|edhi|Edlo|edlo]
            self1 = rp.tile([128, RH1 * nwin], bf16)
            self2 = rp.tile([128, RH2 * nwin], bf16)

            # DRAM tiles
            t1_own = dp.tile([npad, 128], bf16)
            t1_full = dp.tile([NTOT, 128], bf16)
            t2_own = dp.tile([npad, 128], bf16)
            t2_full = dp.tile([NTOT, 128], bf16)

            # ---------------- P0: produce T1 + al1/self1 tables
            for t in range(ntile_x):
                xt = p0.tile([cfg["f_in"], 128], f32, tag="xt")
                nc.sync.dma_start(out=xt[:], in_=x_T[:, t * 128:(t + 1) * 128])
                ps = pm.tile([128, W1C], f32, space="PSUM", tag="pm")
                nc.tensor.matmul(ps[:], lhsT=xt[:], rhs=w1c_sb[:],
                                 start=True, stop=True)
                t1sb = p0.tile([128, 128], bf16, tag="t1sb")
                nc.vector.tensor_copy(t1sb[:, 0:D1], ps[:, 0:D1])
                nc.scalar.activation(t1sb[:, D1:D1 + H], ps[:, D1:D1 + H],
                                     AF.Exp)
                nc.scalar.activation(t1sb[:, D1 + H:D1 + 2 * H],
                                     ps[:, D1:D1 + H], AF.Exp, scale=NEGS)
                nc.vector.memset(t1sb[:, D1 + 2 * H:128], 0.0)
                Edf = p0.tile([128, H], f32, tag="Edf")
                nc.scalar.activation(Edf[:], ps[:, D1 + H:W1C], AF.Exp)
                edf = p0.tile([128, H], f32, tag="edf")
                nc.scalar.activation(edf[:], ps[:, D1 + H:W1C], AF.Exp,
                                     scale=NEGS)
                o = 32 * t
                nc.vector.tensor_copy(al1w[:, o:o + H], Edf[:])
                nc.vector.tensor_copy(al1w[:, o + H:o + 2 * H], edf[:])
                nc.vector.tensor_tensor(out=al1w[:, o + 16:o + 24],
                                        in0=Edf[:], in1=al1w[:, o:o + H],
                                        op=AL.subtract)
                nc.vector.tensor_tensor(out=al1w[:, o + 24:o + 32],
                                        in0=edf[:], in1=al1w[:, o + H:o + 16],
                                        op=AL.subtract)
                # self-loop alpha + rhs
                av = p0.tile([128, H], f32, tag="av")
                nc.vector.tensor_tensor(out=av[:], in0=t1sb[:, D1:D1 + H],
                                        in1=Edf[:], op=AL.mult)
                bv = p0.tile([128, H], f32, tag="bv")
                nc.vector.tensor_tensor(out=bv[:],
                                        in0=t1sb[:, D1 + H:D1 + 2 * H],
                                        in1=edf[:], op=AL.mult)
                so = RH1 * t
                nc.vector.tensor_tensor(out=self1[:, so + D1:so + RH1],
                                        in0=av[:], in1=bv[:], op=AL.max)
                nc.vector.tensor_tensor(
                    out=self1[:, so:so + D1].rearrange("p (a c) -> p a c",
                                                       c=C1),
                    in0=t1sb[:, 0:D1].rearrange("p (a c) -> p a c", c=C1),
                    in1=self1[:, so + D1:so + RH1]
                    .broadcast_to([128, H, C1]),
                    op=AL.mult)
                nc.sync.dma_start(out=t1_own[t * 128:(t + 1) * 128, :],
                                  in_=t1sb[:])

            nc.gpsimd.collective_compute(
                "AllGather", AL.bypass,
                replica_groups=[list(range(nc_))],
                ins=[t1_own.opt()], outs=[t1_full.opt()],
            )

            # ---------------- shared edge pass
            def edge_pass(tfull, alw, selfw, nal, rhw, aws):
                """nal: attention scalars/edge; rhw: rhs width; aws: alw stride."""
                call_i = 0
                calls_by_blk = [[] for _ in range(nblk)]
                for ci, cl in enumerate(st["calls"]):
                    calls_by_blk[cl["blk"]].append(ci)
                for b in range(nblk):
                    ptb = pw.tile([128, wb * RH1], f32, space="PSUM",
                                  tag="pwin")
                    w0 = b * wb
                    for wi in range(wb):
                        w = w0 + wi
                        nc.tensor.matmul(
                            ptb[:, wi * RH1:wi * RH1 + rhw], lhsT=identb[:],
                            rhs=selfw[:, w * rhw:(w + 1) * rhw],
                            start=(wi == 0), stop=False)
                    for ci in calls_by_blk[b]:
                        cl = st["calls"][ci]
                        ni, nt = cl["ni"], cl["ntile"]
                        meta = mp.tile([128, MC], i16, tag="meta")
                        nc.sync.dma_start(out=meta[:],
                                          in_=meta_in[:, ci * MC:(ci + 1) * MC])
                        dcol = meta[:, NI // 16:NI // 16 + nt].bitcast(bf16)
                        drep = dpp.tile([128, NI], bf16, tag="drep")
                        nc.sync.dma_start(
                            out=drep[:, 0:ni],
                            in_=drow_in[:, ci * NI:ci * NI + ni]
                            .to_broadcast([128, ni]))
                        g = gp.tile([128, NI], bf16, tag="g")
                        s_ = cl["shard"]
                        # split the gather across the 4 SWDGE queues so
                        # descriptor generation runs on 4 Q7 pairs in parallel
                        tq = _ceil(nt, 4)
                        base = 0
                        for q in range(4):
                            tc_ = min(tq, nt - base)
                            if tc_ <= 0:
                                break
                            niq = tc_ * 128
                            nc.gpsimd.dma_gather(
                                g[:, base * 128:(base + tc_) * 128]
                                .rearrange("p (b e) -> p b e", e=128),
                                tfull[s_ * GS:(s_ + 1) * GS, :],
                                meta[:, base * 8:(base + tc_) * 8],
                                niq, niq, 128,
                                single_packet=False, queue_num=q)
                            base += tc_
                        call_i += 1
                        # one-hot builds (tensor_scalar with per-partition
                        # scalar keeps APs contiguous -> DVE fast path)
                        dcolf = tvp.tile([128, 32], f32, tag="dcolf")
                        nc.vector.tensor_copy(dcolf[:, 0:nt], dcol)
                        s1 = sp.tile([128, NI], bf16, tag="s1")
                        for t in range(nt):
                            nc.vector.tensor_scalar(
                                out=s1[:, t * 128:(t + 1) * 128],
                                in0=iota_mat[:], scalar1=dcolf[:, t:t + 1],
                                scalar2=None, op0=AL.is_equal)
                        st1 = sp.tile([128, NI], bf16, tag="st1")
                        nc.vector.tensor_tensor(
                            out=st1[:, 0:ni], in0=iotacol_w[:, 0:ni],
                            in1=drep[:, 0:ni], op=AL.is_equal)
                        strads = [t for t, (w1, sdl) in enumerate(cl["tiles"])
                                  if sdl]
                        nstr = len(strads)
                        if nstr:
                            s2s = sp.tile([128, 128 * 8], bf16, tag="s2s")
                            st2s = sp.tile([128, 128 * 8], bf16, tag="st2s")
                            for j, t in enumerate(strads):
                                nc.vector.tensor_scalar(
                                    out=s2s[:, j * 128:(j + 1) * 128],
                                    in0=iota_mat2[:],
                                    scalar1=dcolf[:, t:t + 1],
                                    scalar2=None, op0=AL.is_equal)
                                nc.vector.tensor_tensor(
                                    out=st2s[:, j * 128:(j + 1) * 128],
                                    in0=iotacol2[:],
                                    in1=drep[:, t * 128:(t + 1) * 128],
                                    op=AL.is_equal)
                        sidx = {t: j for j, t in enumerate(strads)}
                        # Ed/ed lookup (hi/lo accumulated in f32 psum)
                        pald = pa.tile([128, 62 * 8], f32, space="PSUM",
                                       tag="pald")
                        na2 = 2 * nal
                        for t, (w1, sdl) in enumerate(cl["tiles"]):
                            po = pald[:, t * na2:(t + 1) * na2]
                            last_t = t == nt - 1
                            nc.tensor.matmul(
                                po, lhsT=st1[:, t * 128:(t + 1) * 128],
                                rhs=alw[:, aws * w1:aws * w1 + na2],
                                start=(t == 0), stop=False)
                            nc.tensor.matmul(
                                po, lhsT=st1[:, t * 128:(t + 1) * 128],
                                rhs=alw[:, aws * w1 + na2:aws * (w1 + 1)],
                                start=False, stop=last_t and not sdl)
                            if sdl:
                                j = sidx[t]
                                sl = st2s[:, j * 128:(j + 1) * 128]
                                nc.tensor.matmul(
                                    po, lhsT=sl,
                                    rhs=alw[:, aws * (w1 + 1):
                                            aws * (w1 + 1) + na2],
                                    start=False, stop=False)
                                nc.tensor.matmul(
                                    po, lhsT=sl,
                                    rhs=alw[:, aws * (w1 + 1) + na2:
                                            aws * (w1 + 2)],
                                    start=False, stop=last_t)
                        # alpha = max(Es*Ed, es*ed); rhs = [msg*alpha | alpha]
                        gv = g[:, 0:ni].rearrange("p (b e) -> p b e", e=128)
                        pv = pald[:, 0:nt * na2].rearrange(
                            "p (b a) -> p b a", a=na2)
                        t1v = tvp.tile([128, 8 * 32], f32, tag="t1v")
                        t2v = tvp.tile([128, 8 * 32], f32, tag="t2v")
                        t1vv = t1v[:, 0:nt * nal].rearrange(
                            "p (b a) -> p b a", a=nal)
                        t2vv = t2v[:, 0:nt * nal].rearrange(
                            "p (b a) -> p b a", a=nal)
                        nc.vector.tensor_tensor(
                            out=t1vv, in0=gv[:, :, D1:D1 + nal],
                            in1=pv[:, :, 0:nal], op=AL.mult)
                        nc.vector.tensor_tensor(
                            out=t2vv, in0=gv[:, :, D1 + nal:D1 + 2 * nal],
                            in1=pv[:, :, nal:na2], op=AL.mult)
                        rhs = rhp.tile([128, RH1 * 32], bf16, tag="rhs")
                        rv = rhs[:, 0:nt * rhw].rearrange(
                            "p (b r) -> p b r", r=rhw)
                        nc.vector.tensor_tensor(
                            out=rv[:, :, D1:D1 + nal], in0=t1vv, in1=t2vv,
                            op=AL.max)
                        cph = D1 // nal
                        nc.vector.tensor_tensor(
                            out=rv[:, :, 0:D1].rearrange(
                                "p b (a c) -> p b a c", c=cph),
                            in0=gv[:, :, 0:D1].rearrange(
                                "p b (a c) -> p b a c", c=cph),
                            in1=rv[:, :, D1:D1 + nal]
                            .broadcast_to([128, nt, nal, cph]),
                            op=AL.mult)
                        # aggregation matmuls
                        for (mm, fstop) in cl["flags"]:
                            _, t, k, w = mm
                            wi = w - w0
                            if k == 0:
                                lhsT = s1[:, t * 128:(t + 1) * 128]
                            else:
                                j = sidx[t]
                                lhsT = s2s[:, j * 128:(j + 1) * 128]
                            nc.tensor.matmul(
                                ptb[:, wi * RH1:wi * RH1 + rhw], lhsT=lhsT,
                                rhs=rhs[:, t * rhw:(t + 1) * rhw],
                                start=False, stop=fstop)
                    yield b, ptb

            # ---------------- L1 pass + epilogue -> T2
            for b, ptb in edge_pass(t1_full, al1w, self1, H, RH1, 32):
                w0 = b * wb
                ptv = ptb[:].rearrange("p (w r) -> p w r", r=RH1)
                rc = ep.tile([128, wb * H], f32, tag="rc1")
                rcv = rc[:].rearrange("p (w a) -> p w a", a=H)
                nc.vector.reciprocal(rcv, ptv[:, :, D1:RH1])
                nc.vector.tensor_tensor(
                    out=rc[:], in0=rc[:],
                    in1=clampc[:].broadcast_to([128, wb * H]), op=AL.min)
                o1 = ep.tile([128, wb * D1], f32, tag="o1")
                o1v = o1[:].rearrange("p (w a c) -> p w a c", a=H, c=C1)
                nc.vector.tensor_tensor(
                    out=o1v,
                    in0=ptv[:, :, 0:D1].rearrange("p w (a c) -> p w a c",
                                                  c=C1),
                    in1=rcv.broadcast_to([128, wb, H, C1]), op=AL.mult)
                b1v = b1m[:].rearrange("p (o c) -> p o c", o=1)
                o1w = o1[:].rearrange("p (w c) -> p w c", c=D1)
                nc.vector.tensor_tensor(
                    out=o1w, in0=o1w,
                    in1=b1v.broadcast_to([128, wb, D1]), op=AL.add)
                ra = ep.tile([128, wb * D1], f32, tag="ra")
                nc.scalar.activation(ra[:], o1[:], AF.Relu)
                rav = ra[:].rearrange("p (w c) -> p w c", c=D1)
                t2sb = ep.tile([128, wb * 128], bf16, tag="t2sb")
                t2v_ = t2sb[:].rearrange("p (w c) -> p w c", c=128)
                for wi in range(wb):
                    nc.vector.tensor_copy(
                        t2sb[:, wi * 128:wi * 128 + D1],
                        ra[:, wi * D1:(wi + 1) * D1])
                nc.vector.memset(t2v_[:, :, D1 + 2:128], 0.0)
                tmp = ep.tile([128, wb * D1], f32, tag="altmp")
                tmpv = tmp[:].rearrange("p (w c) -> p w c", c=D1)
                a2 = ep.tile([128, 2 * wb], f32, tag="a2")
                wsv = wa2sm[:].rearrange("p (o c) -> p o c", o=1)
                nc.vector.tensor_tensor(out=tmpv, in0=rav,
                                        in1=wsv.broadcast_to([128, wb, D1]),
                                        op=AL.mult)
                nc.vector.tensor_reduce(
                    a2[:, 0:wb].rearrange("p (w o) -> p w o", o=1), tmpv,
                    axis=ax_x, op=AL.add)
                wdv = wa2dm[:].rearrange("p (o c) -> p o c", o=1)
                nc.vector.tensor_tensor(out=tmpv, in0=rav,
                                        in1=wdv.broadcast_to([128, wb, D1]),
                                        op=AL.mult)
                nc.vector.tensor_reduce(
                    a2[:, wb:2 * wb].rearrange("p (w o) -> p w o", o=1), tmpv,
                    axis=ax_x, op=AL.add)
                # Es2/es2 into table; Ed2/ed2 hi/lo into al2w; self2
                nc.scalar.activation(t2v_[:, :, D1:D1 + 1], a2[:, 0:wb]
                                     .rearrange("p (w o) -> p w o", o=1), AF.Exp)
                nc.scalar.activation(t2v_[:, :, D1 + 1:D1 + 2], a2[:, 0:wb]
                                     .rearrange("p (w o) -> p w o", o=1), AF.Exp,
                                     scale=NEGS)
                Ed2 = ep.tile([128, 2 * wb], f32, tag="Ed2")
                nc.scalar.activation(Ed2[:, 0:wb], a2[:, wb:2 * wb], AF.Exp)
                nc.scalar.activation(Ed2[:, wb:2 * wb], a2[:, wb:2 * wb],
                                     AF.Exp, scale=NEGS)
                awv = al2w[:, 4 * w0:4 * (w0 + wb)].rearrange(
                    "p (w k) -> p w k", k=4)
                E2v = Ed2[:, 0:wb].rearrange("p (w o) -> p w o", o=1)
                e2v = Ed2[:, wb:2 * wb].rearrange("p (w o) -> p w o", o=1)
                nc.vector.tensor_copy(awv[:, :, 0:1], E2v)
                nc.vector.tensor_copy(awv[:, :, 1:2], e2v)
                nc.vector.tensor_tensor(out=awv[:, :, 2:3], in0=E2v,
                                        in1=awv[:, :, 0:1], op=AL.subtract)
                nc.vector.tensor_tensor(out=awv[:, :, 3:4], in0=e2v,
                                        in1=awv[:, :, 1:2], op=AL.subtract)
                sa = ep.tile([128, 2 * wb], f32, tag="sa")
                nc.vector.tensor_tensor(
                    out=sa[:, 0:wb].rearrange("p (w o) -> p w o", o=1),
                    in0=t2v_[:, :, D1:D1 + 1], in1=E2v, op=AL.mult)
                nc.vector.tensor_tensor(
                    out=sa[:, wb:2 * wb].rearrange("p (w o) -> p w o", o=1),
                    in0=t2v_[:, :, D1 + 1:D1 + 2], in1=e2v, op=AL.mult)
                s2v = self2[:, RH2 * w0:RH2 * (w0 + wb)].rearrange(
                    "p (w r) -> p w r", r=RH2)
                nc.vector.tensor_tensor(
                    out=s2v[:, :, D1:RH2],
                    in0=sa[:, 0:wb].rearrange("p (w o) -> p w o", o=1),
                    in1=sa[:, wb:2 * wb].rearrange("p (w o) -> p w o", o=1),
                    op=AL.max)
                nc.vector.tensor_tensor(
                    out=s2v[:, :, 0:D1], in0=t2v_[:, :, 0:D1],
                    in1=s2v[:, :, D1:RH2].broadcast_to([128, wb, D1]),
                    op=AL.mult)
                nc.sync.dma_start(
                    out=t2_own[w0 * 128:(w0 + wb) * 128, :]
                    .rearrange("(w p) c -> p w c", p=128),
                    in_=t2v_)

            nc.gpsimd.collective_compute(
                "AllGather", AL.bypass,
                replica_groups=[list(range(nc_))],
                ins=[t2_own.opt()], outs=[t2_full.opt()],
            )

            # ---------------- L2 pass + epilogue -> output
            for b, ptb in edge_pass(t2_full, al2w, self2, 1, RH2, 4):
                w0 = b * wb
                ptv = ptb[:].rearrange("p (w r) -> p w r", r=RH1)
                rc = ep.tile([128, wb], f32, tag="rc2")
                nc.vector.reciprocal(rc[:].rearrange("p (w o) -> p w o", o=1),
                                     ptv[:, :, D1:D1 + 1])
                nc.vector.tensor_tensor(
                    out=rc[:], in0=rc[:],
                    in1=clampc[:].broadcast_to([128, wb]), op=AL.min)
                tca = ep.tile([128, wb * D1], bf16, tag="tca")
                tcav = tca[:].rearrange("p (w c) -> p w c", c=D1)
                nc.vector.tensor_copy(tcav, ptv[:, :, 0:D1])
                lg = ep.tile([128, wb * CL], f32, tag="lg")
                for wi in range(wb):
                    trp = ptp.tile([D1, 128], bf16, space="PSUM", tag="trp")
                    nc.tensor.transpose(out=trp[:],
                                        in_=tca[:, wi * D1:(wi + 1) * D1],
                                        identity=identb[:])
                    trs = ep.tile([D1, 128], bf16, tag="trs")
                    nc.vector.tensor_copy(trs[:], trp[:])
                    op2 = ptp.tile([128, CL], f32, space="PSUM", tag="op2")
                    nc.tensor.matmul(op2[:], lhsT=trs[:], rhs=w2b_sb[:],
                                     start=True, stop=True)
                    nc.vector.scalar_tensor_tensor(
                        out=lg[:, wi * CL:(wi + 1) * CL], in0=op2[:],
                        scalar=rc[:, wi:wi + 1], in1=b2m[:],
                        op0=AL.mult, op1=AL.add)
                lgv = lg[:].rearrange("p (w c) -> p w c", c=CL)
                mx = ep.tile([128, wb], f32, tag="mx")
                mxv = mx[:].rearrange("p (w o) -> p w o", o=1)
                nc.vector.tensor_reduce(mxv, lgv, axis=ax_x, op=AL.max)
                nc.vector.tensor_tensor(out=lgv, in0=lgv,
                                        in1=mxv.broadcast_to([128, wb, CL]),
                                        op=AL.subtract)
                exs = ep.tile([128, wb * CL], f32, tag="exs")
                nc.scalar.activation(exs[:], lg[:], AF.Exp)
                sm = ep.tile([128, wb], f32, tag="sm")
                smv = sm[:].rearrange("p (w o) -> p w o", o=1)
                nc.vector.tensor_reduce(
                    smv, exs[:].rearrange("p (w c) -> p w c", c=CL),
                    axis=ax_x, op=AL.add)
                lnm = ep.tile([128, wb], f32, tag="lnm")
                nc.scalar.activation(lnm[:], sm[:], AF.Ln)
                nc.vector.tensor_tensor(
                    out=lgv, in0=lgv,
                    in1=lnm[:].rearrange("p (w o) -> p w o", o=1)
                    .broadcast_to([128, wb, CL]),
                    op=AL.subtract)
                nc.sync.dma_start(
                    out=out_d[w0 * 128:(w0 + wb) * 128, :]
                    .rearrange("(w p) c -> p w c", p=128),
                    in_=lgv)

    nc.compile()
    return nc


def _host_inputs(inputs, cfg, percore):
    x = np.asarray(inputs["x"], np.float32)
    W1 = np.asarray(inputs["W1"], np.float32)
    a_s1 = np.asarray(inputs["a_src1"], np.float32)
    a_d1 = np.asarray(inputs["a_dst1"], np.float32)
    b1 = np.asarray(inputs["b1"], np.float32)
    W2 = np.asarray(inputs["W2"], np.float32)
    a_s2 = np.asarray(inputs["a_src2"], np.float32)
    a_d2 = np.asarray(inputs["a_dst2"], np.float32)
    b2 = np.asarray(inputs["b2"], np.float32)
    H, C1 = cfg["heads"], cfg["hid"]
    D1 = H * C1
    As = np.zeros((D1, H), np.float32)
    Ad = np.zeros((D1, H), np.float32)
    for hd in range(H):
        As[hd * C1:(hd + 1) * C1, hd] = a_s1[hd]
        Ad[hd * C1:(hd + 1) * C1, hd] = a_d1[hd]
    w1cat = np.concatenate([W1, W1 @ As, W1 @ Ad], axis=1)
    wa2s = (W2 @ a_s2[0])[None, :]
    wa2d = (W2 @ a_d2[0])[None, :]
    nsh, npad = cfg["nshard"], cfg["npad"]
    maps = []
    for c in range(cfg["ncores"]):
        xs = x[c * nsh:(c + 1) * nsh]
        xp = np.zeros((npad, cfg["f_in"]), np.float32)
        xp[:xs.shape[0]] = xs
        maps.append(dict(
            x_T=np.ascontiguousarray(xp.T), w1cat=w1cat,
            b1row=b1[None, :], wa2s=wa2s, wa2d=wa2d,
            w2b=W2.astype(BF), b2row=b2[None, :],
            meta_in=percore[c]["meta"], drow_in=percore[c]["drow"],
        ))
    return maps


_CACHE = {}


def kernel(**inputs):
    from concourse import bass_utils

    cfg = FULL_CFG
    ei = np.asarray(inputs["edge_index"])
    src = ei[0].astype(np.int64)
    dst = ei[1].astype(np.int64)

    key = ("full", ei.shape[1])
    if key not in _CACHE:
        st, percore = prep_structure(src, dst, cfg)
        ncobj = build_nc(cfg, st)
        _CACHE[key] = (st, percore, ncobj)
    st, percore, ncobj = _CACHE[key]

    in_maps = _host_inputs(inputs, cfg, percore)
    res = bass_utils.run_bass_kernel_spmd(
        ncobj, in_maps, core_ids=list(range(cfg["ncores"])))
    outs = [res.results[c]["out"][:cfg["nshard"]]
            for c in range(cfg["ncores"])]
    return np.concatenate(outs, axis=0).astype(np.float32)


# revision 29
# speedup vs baseline: 1.3417x; 1.2825x over previous
"""GAT (2-layer, 8-head then 1-head) on 8 Trainium2 NeuronCores.

v2 design: dst-shard nodes across 8 cores (12544-padded shards). Per layer,
each core builds a bf16 node table [npad, 128] = [msg(64) | Es | es | 0...]
where Es = exp(al_src), es = exp(0.2*al_src); AllGathered to all cores.
Attention uses the factorization
    exp(leaky(als+ald)) = max(Es*Ed, es*ed),   Ed = exp(al_dst), ed = exp(.2*al_dst)
so the per-edge work is two multiplies and a max; Ed/ed come from a per-window
resident table looked up on the PE via one-hot S^T matmuls (bf16 hi/lo pairs
accumulated in f32 PSUM for full precision).

Edges are dst-owned, grouped by (block of 7 windows, src-super-shard of 25088
rows, window) with core-common structure; h[src] rows are fetched with ONE
dma_gather per (block, super-shard) stream (~3.9k edges) to amortize the ~5us
SWDGE fixed cost. One-hot S matrices are built on DVE (st1 hits the 2x 16-bit
path against a constant iotacol_wide). Aggregation matmuls accumulate
[msg*alpha | alpha] into per-window PSUM slices packed 7-to-a-bank.
Self-loops are folded in as per-window SBUF-resident rhs computed during the
producer phase and added via one identity matmul per window. Layer 2
aggregates relu1 and applies W2 after aggregation; 1/denominator is applied
after the W2 matmul (row scalars commute).
"""
import sys
import numpy as np

sys.path.insert(0, "/opt/trn_rl_repo")
import ml_dtypes

BF = ml_dtypes.bfloat16

N = 100000
F_IN = 128
HID = 8
HEADS = 8
CLASSES = 40
NEG = 0.2
NC = 8

FULL_CFG = dict(
    ncores=8, nshard=12500, npad=12544, wb=7, ngsh=4, ni_max=3968,
    f_in=128, heads=8, hid=8, classes=40, neg=0.2,
)


def _ceil(a, b):
    return -(-a // b)


# ---------------------------------------------------------------- host prep
def prep_structure(src, dst, cfg):
    """Build core-common call/tile/matmul structure + per-core arrays.

    Returns (st, percore): st holds the shared program structure, percore[c]
    holds the packed meta (idx|dcol) and drow arrays for core c.
    """
    nc_, nsh, npad, wb = cfg["ncores"], cfg["nshard"], cfg["npad"], cfg["wb"]
    ngsh, NI = cfg["ngsh"], cfg["ni_max"]
    nwin = npad // 128                      # 98
    nblk = _ceil(nwin, wb)                  # 14
    assert nwin % wb == 0, "code assumes full blocks"
    gsz = (nc_ * npad) // ngsh              # 25088 rows per gather super-shard
    padrow = nsh                            # zero-ish row within super-shard

    core = dst // nsh
    dstl = dst % nsh
    g_row = (src // nsh) * npad + (src % nsh)
    gsh = g_row // gsz
    gloc = g_row % gsz
    win = dstl // 128
    blk = win // wb

    counts = np.zeros((nc_, nblk, ngsh, nwin), dtype=np.int64)
    np.add.at(counts, (core, blk, gsh, win), 1)
    common = counts.max(axis=0)             # [nblk, ngsh, nwin]
    for b in range(nblk):
        assert common[b, :, b * wb:(b + 1) * wb].min() >= 128, \
            "tile could span >2 windows"

    # per-core edge arrays sorted by (block, gshard, window)
    percore_edges = []
    for c in range(nc_):
        m = core == c
        key = (blk[m].astype(np.int64) * ngsh + gsh[m]) * nwin + win[m]
        o = np.argsort(key, kind="stable")
        percore_edges.append((key[o], gloc[m][o], dstl[m][o], win[m][o]))

    calls = []
    pc_meta = [[] for _ in range(nc_)]
    pc_drow = [[] for _ in range(nc_)]
    win_mms = {}                            # (b, w) -> list of mm ids
    MC = NI // 16 + 32                      # meta cols (idx | dcol-as-i16)

    for b in range(nblk):
        for s in range(ngsh):
            streams = []
            for c in range(nc_):
                kk, sl, dl, wn = percore_edges[c]
                segs = []
                for w in range(b * wb, (b + 1) * wb):
                    kval = (b * ngsh + s) * nwin + w
                    lo = np.searchsorted(kk, kval, "left")
                    hi = np.searchsorted(kk, kval, "right")
                    n_common = common[b, s, w]
                    seg_s = np.full(n_common, padrow, dtype=np.int64)
                    seg_w = np.full(n_common, w, dtype=np.int64)
                    seg_d = np.full(n_common, -1, dtype=np.int64)
                    seg_s[: hi - lo] = sl[lo:hi]
                    seg_w[: hi - lo] = wn[lo:hi]
                    seg_d[: hi - lo] = dl[lo:hi]
                    segs.append(np.stack([seg_s, seg_w, seg_d]))
                streams.append(np.concatenate(segs, axis=1))
            L = streams[0].shape[1]
            pos = 0
            while pos < L:
                ni_real = min(NI, L - pos)
                ni = _ceil(ni_real, 128) * 128
                ntile = ni // 128
                wseg = streams[0][1][pos:pos + ni_real]
                tiles = []
                for t in range(ntile):
                    a, z = t * 128, min((t + 1) * 128, ni_real)
                    if a < ni_real:
                        tw = wseg[a:z]
                        w1 = int(tw.min())
                        assert int(tw.max()) - w1 <= 1, "tile spans >2 windows"
                        straddle = int(tw.max()) > w1
                    else:
                        w1, straddle = int(wseg[-1]), False
                    tiles.append((w1, straddle))
                cid = len(calls)
                mms = []
                for t, (w1, straddle) in enumerate(tiles):
                    for k in ([0, 1] if straddle else [0]):
                        mm_id = (cid, t, k, w1 + k)
                        win_mms.setdefault((b, w1 + k), []).append(mm_id)
                        mms.append(mm_id)
                calls.append(dict(blk=b, shard=s, ni=ni, ntile=ntile,
                                  tiles=tiles, mms=mms))
                for c in range(nc_):
                    ss, ww, dd = streams[c]
                    sl_call = np.full(ni, padrow, dtype=np.int64)
                    rel_call = np.full(ni, 300.0, dtype=np.float64)
                    nreal = min(ni_real, L - pos)
                    sl_call[:nreal] = ss[pos:pos + nreal]
                    for t in range(ntile):
                        a, z = t * 128, min((t + 1) * 128, nreal)
                        if a >= nreal:
                            break
                        w1 = tiles[t][0]
                        dv = dd[pos + a:pos + z]
                        wv = ww[pos + a:pos + z]
                        rel = (wv - w1) * 128 + (dv - wv * 128)
                        rel = np.where(dv < 0, 300.0, rel)
                        rel_call[a:z] = rel
                    # meta: idx wrapped [16, ni/16] tiled x8, then dcol bf16 bits
                    meta = np.zeros((128, MC), np.int16)
                    iw = sl_call.reshape(ni // 16, 16).T.astype(np.int16)
                    meta[:, 0:ni // 16] = np.tile(iw, (8, 1))
                    dcol = rel_call.reshape(ntile, 128).T.astype(BF)
                    meta[:, NI // 16:NI // 16 + ntile] = dcol.view(np.int16)
                    pc_meta[c].append(meta)
                    pc_drow[c].append(rel_call.astype(BF))
                pos += ni_real

    # One PSUM accumulation group per block bank: the first self matmul
    # starts it (start=True zeroes the whole 2KB bank), the absolute last
    # aggregation matmul of the block stops it.
    last_mm_of_blk = {}
    for cl in calls:
        if cl["mms"]:
            last_mm_of_blk[cl["blk"]] = cl["mms"][-1]
    stopset = set(last_mm_of_blk.values())
    assert len(stopset) == nblk, "every block must have edge matmuls"
    for cl in calls:
        cl["flags"] = [(m, m in stopset) for m in cl["mms"]]

    ncalls = len(calls)
    meta_t = [np.zeros((128, MC * ncalls), np.int16) for _ in range(nc_)]
    drow_t = [np.full((1, NI * ncalls), 300.0, BF) for _ in range(nc_)]
    for c in range(nc_):
        for i in range(ncalls):
            meta_t[c][:, i * MC:(i + 1) * MC] = pc_meta[c][i]
            ni = calls[i]["ni"]
            drow_t[c][0, i * NI:i * NI + ni] = pc_drow[c][i]

    st = dict(calls=calls, nwin=nwin, nblk=nblk, ncalls=ncalls, MC=MC)
    percore = [dict(meta=meta_t[c], drow=drow_t[c]) for c in range(nc_)]
    return st, percore


# ---------------------------------------------------------------- program
def build_nc(cfg, st):
    import concourse.bass as bass
    import concourse.bacc as bacc
    import concourse.tile as tile
    import concourse.mybir as mybir
    from concourse.masks import make_identity

    bf16, f32 = mybir.dt.bfloat16, mybir.dt.float32
    i16, i32 = mybir.dt.int16, mybir.dt.int32
    AL = mybir.AluOpType
    AF = mybir.ActivationFunctionType
    ax_x = mybir.AxisListType.X

    nc_, nsh, npad, wb = cfg["ncores"], cfg["nshard"], cfg["npad"], cfg["wb"]
    ngsh, NI = cfg["ngsh"], cfg["ni_max"]
    H, C1, CL = cfg["heads"], cfg["hid"], cfg["classes"]
    D1 = H * C1                      # 64
    NEGS = cfg["neg"]
    nwin, nblk, ncalls = st["nwin"], st["nblk"], st["ncalls"]
    MC = st["MC"]
    NTOT = nc_ * npad
    GS = NTOT // ngsh                # 25088
    ntile_x = npad // 128            # 98
    RH1 = D1 + H                     # 72
    RH2 = D1 + 1                     # 65
    W1C = D1 + 2 * H                 # 80

    nc = bacc.Bacc("TRN2", target_bir_lowering=False, debug=False,
                   enable_asserts=False, num_devices=nc_, num_swdge_queues=4)

    # ---- I/O
    x_T = nc.dram_tensor("x_T", [cfg["f_in"], npad], f32, kind="ExternalInput")
    w1cat = nc.dram_tensor("w1cat", [cfg["f_in"], W1C], f32,
                           kind="ExternalInput")
    b1row = nc.dram_tensor("b1row", [1, D1], f32, kind="ExternalInput")
    wa2s = nc.dram_tensor("wa2s", [1, D1], f32, kind="ExternalInput")
    wa2d = nc.dram_tensor("wa2d", [1, D1], f32, kind="ExternalInput")
    w2b = nc.dram_tensor("w2b", [D1, CL], bf16, kind="ExternalInput")
    b2row = nc.dram_tensor("b2row", [1, CL], f32, kind="ExternalInput")
    meta_in = nc.dram_tensor("meta_in", [128, MC * ncalls], i16,
                             kind="ExternalInput")
    drow_in = nc.dram_tensor("drow_in", [1, NI * ncalls], bf16,
                             kind="ExternalInput")
    out_d = nc.dram_tensor("out", [npad, CL], f32, kind="ExternalOutput")

    with tile.TileContext(nc) as tc:
        with (
            tc.tile_pool(name="const", bufs=1) as cpool,
            tc.tile_pool(name="res", bufs=1) as rp,
            tc.tile_pool(name="p0", bufs=3) as p0,
            tc.tile_pool(name="meta", bufs=6) as mp,
            tc.tile_pool(name="drp", bufs=3) as dpp,
            tc.tile_pool(name="gpool", bufs=5) as gp,
            tc.tile_pool(name="spool", bufs=2) as sp,
            tc.tile_pool(name="rhsp", bufs=3) as rhp,
            tc.tile_pool(name="tv", bufs=3) as tvp,
            tc.tile_pool(name="epi", bufs=2) as ep,
            tc.tile_pool(name="pwin", bufs=2, space="PSUM") as pw,
            tc.tile_pool(name="pald", bufs=2, space="PSUM") as pa,
            tc.tile_pool(name="pm", bufs=2, space="PSUM") as pm,
            tc.tile_pool(name="ptr", bufs=1, space="PSUM") as ptp,
            tc.tile_pool(name="dram", bufs=1, space="DRAM") as dp,
        ):
            # ---------- constants
            ident = cpool.tile([128, 128], f32)
            make_identity(nc, ident[:])
            identb = cpool.tile([128, 128], bf16)
            nc.vector.tensor_copy(identb[:], ident[:])
            iota_i = cpool.tile([128, 128], i32)
            nc.gpsimd.iota(iota_i[:], pattern=[[1, 128]], base=0,
                           channel_multiplier=0)
            iota_mat = cpool.tile([128, 128], bf16)
            nc.vector.tensor_copy(iota_mat[:], iota_i[:])
            iota_mat2 = cpool.tile([128, 128], bf16)
            nc.vector.tensor_scalar_add(iota_mat2[:], iota_mat[:], 128.0)
            ic_i = cpool.tile([128, 1], i32)
            nc.gpsimd.iota(ic_i[:], pattern=[[0, 1]], base=0,
                           channel_multiplier=1)
            iota_col = cpool.tile([128, 1], bf16)
            nc.vector.tensor_copy(iota_col[:], ic_i[:])
            iotacol_w = cpool.tile([128, NI], bf16)
            nc.vector.tensor_copy(iotacol_w[:],
                                  iota_col[:].broadcast_to([128, NI]))
            iotacol2 = cpool.tile([128, 128], bf16)
            nc.vector.tensor_copy(
                iotacol2[:],
                iota_col[:].broadcast_to([128, 128]))
            nc.vector.tensor_scalar_add(iotacol2[:], iotacol2[:], 128.0)
            b1m = cpool.tile([128, D1], f32)
            nc.sync.dma_start(out=b1m[:], in_=b1row[:].to_broadcast([128, D1]))
            wa2sm = cpool.tile([128, D1], f32)
            nc.sync.dma_start(out=wa2sm[:], in_=wa2s[:].to_broadcast([128, D1]))
            wa2dm = cpool.tile([128, D1], f32)
            nc.sync.dma_start(out=wa2dm[:], in_=wa2d[:].to_broadcast([128, D1]))
            b2m = cpool.tile([128, CL], f32)
            nc.sync.dma_start(out=b2m[:], in_=b2row[:].to_broadcast([128, CL]))
            w1c_sb = cpool.tile([cfg["f_in"], W1C], f32)
            nc.sync.dma_start(out=w1c_sb[:], in_=w1cat[:])
            w2b_sb = cpool.tile([D1, CL], bf16)
            nc.sync.dma_start(out=w2b_sb[:], in_=w2b[:])
            clampc = cpool.tile([128, 1], f32)
            nc.vector.memset(clampc[:], 1e30)
            iotat_w = cpool.tile([128, NI], bf16)
            nc.vector.tensor_copy(
                iotat_w[:].rearrange("p (b n) -> p b n", n=128),
                iota_mat[:].rearrange("p (o n) -> p o n", o=1)
                .broadcast_to([128, NI // 128, 128]))

            # resident: Ed/ed window tables (hi/lo bf16) + self-loop rhs
            al1w = rp.tile([128, 32 * nwin], bf16)  # [Edhi8|edhi8|Edlo8|edlo8]
            al2w = rp.tile([128, 4 * nwin], bf16)   # [Edhi|edhi|Edlo|edlo]
            self1 = rp.tile([128, RH1 * nwin], bf16)
            self2 = rp.tile([128, RH2 * nwin], bf16)

            # DRAM tiles
            t1_own = dp.tile([npad, 128], bf16)
            t1_full = dp.tile([NTOT, 128], bf16)
            t2_own = dp.tile([npad, 128], bf16)
            t2_full = dp.tile([NTOT, 128], bf16)

            # ---------------- P0: produce T1 + al1/self1 tables
            for t in range(ntile_x):
                xt = p0.tile([cfg["f_in"], 128], f32, tag="xt")
                nc.sync.dma_start(out=xt[:], in_=x_T[:, t * 128:(t + 1) * 128])
                ps = pm.tile([128, W1C], f32, space="PSUM", tag="pm")
                nc.tensor.matmul(ps[:], lhsT=xt[:], rhs=w1c_sb[:],
                                 start=True, stop=True)
                t1sb = p0.tile([128, 128], bf16, tag="t1sb")
                nc.vector.tensor_copy(t1sb[:, 0:D1], ps[:, 0:D1])
                nc.scalar.activation(t1sb[:, D1:D1 + H], ps[:, D1:D1 + H],
                                     AF.Exp)
                nc.scalar.activation(t1sb[:, D1 + H:D1 + 2 * H],
                                     ps[:, D1:D1 + H], AF.Exp, scale=NEGS)
                nc.vector.memset(t1sb[:, D1 + 2 * H:128], 0.0)
                Edf = p0.tile([128, H], f32, tag="Edf")
                nc.scalar.activation(Edf[:], ps[:, D1 + H:W1C], AF.Exp)
                edf = p0.tile([128, H], f32, tag="edf")
                nc.scalar.activation(edf[:], ps[:, D1 + H:W1C], AF.Exp,
                                     scale=NEGS)
                o = 32 * t
                nc.vector.tensor_copy(al1w[:, o:o + H], Edf[:])
                nc.vector.tensor_copy(al1w[:, o + H:o + 2 * H], edf[:])
                nc.vector.tensor_tensor(out=al1w[:, o + 16:o + 24],
                                        in0=Edf[:], in1=al1w[:, o:o + H],
                                        op=AL.subtract)
                nc.vector.tensor_tensor(out=al1w[:, o + 24:o + 32],
                                        in0=edf[:], in1=al1w[:, o + H:o + 16],
                                        op=AL.subtract)
                # self-loop alpha + rhs
                av = p0.tile([128, H], f32, tag="av")
                nc.vector.tensor_tensor(out=av[:], in0=t1sb[:, D1:D1 + H],
                                        in1=Edf[:], op=AL.mult)
                bv = p0.tile([128, H], f32, tag="bv")
                nc.vector.tensor_tensor(out=bv[:],
                                        in0=t1sb[:, D1 + H:D1 + 2 * H],
                                        in1=edf[:], op=AL.mult)
                so = RH1 * t
                nc.vector.tensor_tensor(out=self1[:, so + D1:so + RH1],
                                        in0=av[:], in1=bv[:], op=AL.max)
                nc.vector.tensor_tensor(
                    out=self1[:, so:so + D1].rearrange("p (a c) -> p a c",
                                                       c=C1),
                    in0=t1sb[:, 0:D1].rearrange("p (a c) -> p a c", c=C1),
                    in1=self1[:, so + D1:so + RH1]
                    .broadcast_to([128, H, C1]),
                    op=AL.mult)
                nc.sync.dma_start(out=t1_own[t * 128:(t + 1) * 128, :],
                                  in_=t1sb[:])

            nc.gpsimd.collective_compute(
                "AllGather", AL.bypass,
                replica_groups=[list(range(nc_))],
                ins=[t1_own.opt()], outs=[t1_full.opt()],
            )

            # ---------------- shared edge pass
            def edge_pass(tfull, alw, selfw, nal, rhw, aws):
                """nal: attention scalars/edge; rhw: rhs width; aws: alw stride."""
                call_i = 0
                calls_by_blk = [[] for _ in range(nblk)]
                for ci, cl in enumerate(st["calls"]):
                    calls_by_blk[cl["blk"]].append(ci)
                for b in range(nblk):
                    ptb = pw.tile([128, wb * RH1], f32, space="PSUM",
                                  tag="pwin")
                    w0 = b * wb
                    for wi in range(wb):
                        w = w0 + wi
                        nc.tensor.matmul(
                            ptb[:, wi * RH1:wi * RH1 + rhw], lhsT=identb[:],
                            rhs=selfw[:, w * rhw:(w + 1) * rhw],
                            start=(wi == 0), stop=False)
                    for ci in calls_by_blk[b]:
                        cl = st["calls"][ci]
                        ni, nt = cl["ni"], cl["ntile"]
                        meta = mp.tile([128, MC], i16, tag="meta")
                        nc.sync.dma_start(out=meta[:],
                                          in_=meta_in[:, ci * MC:(ci + 1) * MC])
                        dcol = meta[:, NI // 16:NI // 16 + nt].bitcast(bf16)
                        drep = dpp.tile([128, NI], bf16, tag="drep")
                        nc.sync.dma_start(
                            out=drep[:, 0:ni],
                            in_=drow_in[:, ci * NI:ci * NI + ni]
                            .to_broadcast([128, ni]))
                        g = gp.tile([128, NI], bf16, tag="g")
                        s_ = cl["shard"]
                        # split the gather across the 4 SWDGE queues so
                        # descriptor generation runs on 4 Q7 pairs in parallel
                        tq = _ceil(nt, 4)
                        base = 0
                        for q in range(4):
                            tc_ = min(tq, nt - base)
                            if tc_ <= 0:
                                break
                            niq = tc_ * 128
                            nc.gpsimd.dma_gather(
                                g[:, base * 128:(base + tc_) * 128]
                                .rearrange("p (b e) -> p b e", e=128),
                                tfull[s_ * GS:(s_ + 1) * GS, :],
                                meta[:, base * 8:(base + tc_) * 8],
                                niq, niq, 128,
                                single_packet=False, queue_num=q)
                            base += tc_
                        call_i += 1
                        # one-hot builds (single broadcast operand against a
                        # contiguous tiled-iota constant)
                        s1 = sp.tile([128, NI], bf16, tag="s1")
                        nc.vector.tensor_tensor(
                            out=s1[:, 0:ni].rearrange("p (b n) -> p b n",
                                                      n=128),
                            in0=dcol.broadcast_to([128, nt, 128]),
                            in1=iotat_w[:, 0:ni]
                            .rearrange("p (b n) -> p b n", n=128),
                            op=AL.is_equal)
                        st1 = sp.tile([128, NI], bf16, tag="st1")
                        nc.vector.tensor_tensor(
                            out=st1[:, 0:ni], in0=iotacol_w[:, 0:ni],
                            in1=drep[:, 0:ni], op=AL.is_equal)
                        strads = [t for t, (w1, sdl) in enumerate(cl["tiles"])
                                  if sdl]
                        nstr = len(strads)
                        if nstr:
                            s2s = sp.tile([128, 128 * 8], bf16, tag="s2s")
                            st2s = sp.tile([128, 128 * 8], bf16, tag="st2s")
                            for j, t in enumerate(strads):
                                nc.vector.tensor_tensor(
                                    out=s2s[:, j * 128:(j + 1) * 128],
                                    in0=dcol[:, t:t + 1]
                                    .broadcast_to([128, 128]),
                                    in1=iota_mat2[:], op=AL.is_equal)
                                nc.vector.tensor_tensor(
                                    out=st2s[:, j * 128:(j + 1) * 128],
                                    in0=iotacol2[:],
                                    in1=drep[:, t * 128:(t + 1) * 128],
                                    op=AL.is_equal)
                        sidx = {t: j for j, t in enumerate(strads)}
                        # Ed/ed lookup (hi/lo accumulated in f32 psum)
                        pald = pa.tile([128, 62 * 8], f32, space="PSUM",
                                       tag="pald")
                        na2 = 2 * nal
                        for t, (w1, sdl) in enumerate(cl["tiles"]):
                            po = pald[:, t * na2:(t + 1) * na2]
                            last_t = t == nt - 1
                            nc.tensor.matmul(
                                po, lhsT=st1[:, t * 128:(t + 1) * 128],
                                rhs=alw[:, aws * w1:aws * w1 + na2],
                                start=(t == 0), stop=False)
                            nc.tensor.matmul(
                                po, lhsT=st1[:, t * 128:(t + 1) * 128],
                                rhs=alw[:, aws * w1 + na2:aws * (w1 + 1)],
                                start=False, stop=last_t and not sdl)
                            if sdl:
                                j = sidx[t]
                                sl = st2s[:, j * 128:(j + 1) * 128]
                                nc.tensor.matmul(
                                    po, lhsT=sl,
                                    rhs=alw[:, aws * (w1 + 1):
                                            aws * (w1 + 1) + na2],
                                    start=False, stop=False)
                                nc.tensor.matmul(
                                    po, lhsT=sl,
                                    rhs=alw[:, aws * (w1 + 1) + na2:
                                            aws * (w1 + 2)],
                                    start=False, stop=last_t)
                        # alpha = max(Es*Ed, es*ed); rhs = [msg*alpha | alpha]
                        gv = g[:, 0:ni].rearrange("p (b e) -> p b e", e=128)
                        pv = pald[:, 0:nt * na2].rearrange(
                            "p (b a) -> p b a", a=na2)
                        t1v = tvp.tile([128, 8 * 32], f32, tag="t1v")
                        t2v = tvp.tile([128, 8 * 32], f32, tag="t2v")
                        t1vv = t1v[:, 0:nt * nal].rearrange(
                            "p (b a) -> p b a", a=nal)
                        t2vv = t2v[:, 0:nt * nal].rearrange(
                            "p (b a) -> p b a", a=nal)
                        nc.vector.tensor_tensor(
                            out=t1vv, in0=gv[:, :, D1:D1 + nal],
                            in1=pv[:, :, 0:nal], op=AL.mult)
                        nc.vector.tensor_tensor(
                            out=t2vv, in0=gv[:, :, D1 + nal:D1 + 2 * nal],
                            in1=pv[:, :, nal:na2], op=AL.mult)
                        rhs = rhp.tile([128, RH1 * 32], bf16, tag="rhs")
                        rv = rhs[:, 0:nt * rhw].rearrange(
                            "p (b r) -> p b r", r=rhw)
                        nc.vector.tensor_tensor(
                            out=rv[:, :, D1:D1 + nal], in0=t1vv, in1=t2vv,
                            op=AL.max)
                        cph = D1 // nal
                        nc.vector.tensor_tensor(
                            out=rv[:, :, 0:D1].rearrange(
                                "p b (a c) -> p b a c", c=cph),
                            in0=gv[:, :, 0:D1].rearrange(
                                "p b (a c) -> p b a c", c=cph),
                            in1=rv[:, :, D1:D1 + nal]
                            .broadcast_to([128, nt, nal, cph]),
                            op=AL.mult)
                        # aggregation matmuls
                        for (mm, fstop) in cl["flags"]:
                            _, t, k, w = mm
                            wi = w - w0
                            if k == 0:
                                lhsT = s1[:, t * 128:(t + 1) * 128]
                            else:
                                j = sidx[t]
                                lhsT = s2s[:, j * 128:(j + 1) * 128]
                            nc.tensor.matmul(
                                ptb[:, wi * RH1:wi * RH1 + rhw], lhsT=lhsT,
                                rhs=rhs[:, t * rhw:(t + 1) * rhw],
                                start=False, stop=fstop)
                    yield b, ptb

            # ---------------- L1 pass + epilogue -> T2
            for b, ptb in edge_pass(t1_full, al1w, self1, H, RH1, 32):
                w0 = b * wb
                ptv = ptb[:].rearrange("p (w r) -> p w r", r=RH1)
                den = ep.tile([128, wb * H], f32, tag="den1")
                nc.vector.tensor_copy(
                    den[:].rearrange("p (w a) -> p w a", a=H),
                    ptv[:, :, D1:RH1])
                rc = ep.tile([128, wb * H], f32, tag="rc1")
                rcv = rc[:].rearrange("p (w a) -> p w a", a=H)
                nc.vector.reciprocal(rc[:], den[:])
                nc.vector.tensor_tensor(
                    out=rc[:], in0=rc[:],
                    in1=clampc[:].broadcast_to([128, wb * H]), op=AL.min)
                o1 = ep.tile([128, wb * D1], f32, tag="o1")
                o1v = o1[:].rearrange("p (w a c) -> p w a c", a=H, c=C1)
                nc.vector.tensor_tensor(
                    out=o1v,
                    in0=ptv[:, :, 0:D1].rearrange("p w (a c) -> p w a c",
                                                  c=C1),
                    in1=rcv.broadcast_to([128, wb, H, C1]), op=AL.mult)
                b1v = b1m[:].rearrange("p (o c) -> p o c", o=1)
                o1w = o1[:].rearrange("p (w c) -> p w c", c=D1)
                nc.vector.tensor_tensor(
                    out=o1w, in0=o1w,
                    in1=b1v.broadcast_to([128, wb, D1]), op=AL.add)
                ra = ep.tile([128, wb * D1], f32, tag="ra")
                nc.scalar.activation(ra[:], o1[:], AF.Relu)
                rav = ra[:].rearrange("p (w c) -> p w c", c=D1)
                t2sb = ep.tile([128, wb * 128], bf16, tag="t2sb")
                t2v_ = t2sb[:].rearrange("p (w c) -> p w c", c=128)
                for wi in range(wb):
                    nc.vector.tensor_copy(
                        t2sb[:, wi * 128:wi * 128 + D1],
                        ra[:, wi * D1:(wi + 1) * D1])
                nc.vector.memset(t2v_[:, :, D1 + 2:128], 0.0)
                tmp = ep.tile([128, wb * D1], f32, tag="altmp")
                tmpv = tmp[:].rearrange("p (w c) -> p w c", c=D1)
                a2 = ep.tile([128, 2 * wb], f32, tag="a2")
                wsv = wa2sm[:].rearrange("p (o c) -> p o c", o=1)
                nc.vector.tensor_tensor(out=tmpv, in0=rav,
                                        in1=wsv.broadcast_to([128, wb, D1]),
                                        op=AL.mult)
                nc.vector.tensor_reduce(
                    a2[:, 0:wb].rearrange("p (w o) -> p w o", o=1), tmpv,
                    axis=ax_x, op=AL.add)
                wdv = wa2dm[:].rearrange("p (o c) -> p o c", o=1)
                nc.vector.tensor_tensor(out=tmpv, in0=rav,
                                        in1=wdv.broadcast_to([128, wb, D1]),
                                        op=AL.mult)
                nc.vector.tensor_reduce(
                    a2[:, wb:2 * wb].rearrange("p (w o) -> p w o", o=1), tmpv,
                    axis=ax_x, op=AL.add)
                # Es2/es2 into table; Ed2/ed2 hi/lo into al2w; self2
                nc.scalar.activation(t2v_[:, :, D1:D1 + 1], a2[:, 0:wb]
                                     .rearrange("p (w o) -> p w o", o=1), AF.Exp)
                nc.scalar.activation(t2v_[:, :, D1 + 1:D1 + 2], a2[:, 0:wb]
                                     .rearrange("p (w o) -> p w o", o=1), AF.Exp,
                                     scale=NEGS)
                Ed2 = ep.tile([128, 2 * wb], f32, tag="Ed2")
                nc.scalar.activation(Ed2[:, 0:wb], a2[:, wb:2 * wb], AF.Exp)
                nc.scalar.activation(Ed2[:, wb:2 * wb], a2[:, wb:2 * wb],
                                     AF.Exp, scale=NEGS)
                awv = al2w[:, 4 * w0:4 * (w0 + wb)].rearrange(
                    "p (w k) -> p w k", k=4)
                E2v = Ed2[:, 0:wb].rearrange("p (w o) -> p w o", o=1)
                e2v = Ed2[:, wb:2 * wb].rearrange("p (w o) -> p w o", o=1)
                nc.vector.tensor_copy(awv[:, :, 0:1], E2v)
                nc.vector.tensor_copy(awv[:, :, 1:2], e2v)
                nc.vector.tensor_tensor(out=awv[:, :, 2:3], in0=E2v,
                                        in1=awv[:, :, 0:1], op=AL.subtract)
                nc.vector.tensor_tensor(out=awv[:, :, 3:4], in0=e2v,
                                        in1=awv[:, :, 1:2], op=AL.subtract)
                sa = ep.tile([128, 2 * wb], f32, tag="sa")
                nc.vector.tensor_tensor(
                    out=sa[:, 0:wb].rearrange("p (w o) -> p w o", o=1),
                    in0=t2v_[:, :, D1:D1 + 1], in1=E2v, op=AL.mult)
                nc.vector.tensor_tensor(
                    out=sa[:, wb:2 * wb].rearrange("p (w o) -> p w o", o=1),
                    in0=t2v_[:, :, D1 + 1:D1 + 2], in1=e2v, op=AL.mult)
                s2v = self2[:, RH2 * w0:RH2 * (w0 + wb)].rearrange(
                    "p (w r) -> p w r", r=RH2)
                nc.vector.tensor_tensor(
                    out=s2v[:, :, D1:RH2],
                    in0=sa[:, 0:wb].rearrange("p (w o) -> p w o", o=1),
                    in1=sa[:, wb:2 * wb].rearrange("p (w o) -> p w o", o=1),
                    op=AL.max)
                nc.vector.tensor_tensor(
                    out=s2v[:, :, 0:D1], in0=t2v_[:, :, 0:D1],
                    in1=s2v[:, :, D1:RH2].broadcast_to([128, wb, D1]),
                    op=AL.mult)
                nc.sync.dma_start(
                    out=t2_own[w0 * 128:(w0 + wb) * 128, :]
                    .rearrange("(w p) c -> p w c", p=128),
                    in_=t2v_)

            nc.gpsimd.collective_compute(
                "AllGather", AL.bypass,
                replica_groups=[list(range(nc_))],
                ins=[t2_own.opt()], outs=[t2_full.opt()],
            )

            # ---------------- L2 pass + epilogue -> output
            for b, ptb in edge_pass(t2_full, al2w, self2, 1, RH2, 4):
                w0 = b * wb
                ptv = ptb[:].rearrange("p (w r) -> p w r", r=RH1)
                den2 = ep.tile([128, wb], f32, tag="den2")
                nc.vector.tensor_copy(
                    den2[:].rearrange("p (w o) -> p w o", o=1),
                    ptv[:, :, D1:D1 + 1])
                rc = ep.tile([128, wb], f32, tag="rc2")
                nc.vector.reciprocal(rc[:], den2[:])
                nc.vector.tensor_tensor(
                    out=rc[:], in0=rc[:],
                    in1=clampc[:].broadcast_to([128, wb]), op=AL.min)
                tca = ep.tile([128, wb * D1], bf16, tag="tca")
                tcav = tca[:].rearrange("p (w c) -> p w c", c=D1)
                nc.vector.tensor_copy(tcav, ptv[:, :, 0:D1])
                lg = ep.tile([128, wb * CL], f32, tag="lg")
                for wi in range(wb):
                    trp = ptp.tile([D1, 128], bf16, space="PSUM", tag="trp")
                    nc.tensor.transpose(out=trp[:],
                                        in_=tca[:, wi * D1:(wi + 1) * D1],
                                        identity=identb[:])
                    trs = ep.tile([D1, 128], bf16, tag="trs")
                    nc.vector.tensor_copy(trs[:], trp[:])
                    op2 = ptp.tile([128, CL], f32, space="PSUM", tag="op2")
                    nc.tensor.matmul(op2[:], lhsT=trs[:], rhs=w2b_sb[:],
                                     start=True, stop=True)
                    nc.vector.scalar_tensor_tensor(
                        out=lg[:, wi * CL:(wi + 1) * CL], in0=op2[:],
                        scalar=rc[:, wi:wi + 1], in1=b2m[:],
                        op0=AL.mult, op1=AL.add)
                lgv = lg[:].rearrange("p (w c) -> p w c", c=CL)
                mx = ep.tile([128, wb], f32, tag="mx")
                mxv = mx[:].rearrange("p (w o) -> p w o", o=1)
                nc.vector.tensor_reduce(mxv, lgv, axis=ax_x, op=AL.max)
                nc.vector.tensor_tensor(out=lgv, in0=lgv,
                                        in1=mxv.broadcast_to([128, wb, CL]),
                                        op=AL.subtract)
                exs = ep.tile([128, wb * CL], f32, tag="exs")
                nc.scalar.activation(exs[:], lg[:], AF.Exp)
                sm = ep.tile([128, wb], f32, tag="sm")
                smv = sm[:].rearrange("p (w o) -> p w o", o=1)
                nc.vector.tensor_reduce(
                    smv, exs[:].rearrange("p (w c) -> p w c", c=CL),
                    axis=ax_x, op=AL.add)
                lnm = ep.tile([128, wb], f32, tag="lnm")
                nc.scalar.activation(lnm[:], sm[:], AF.Ln)
                nc.vector.tensor_tensor(
                    out=lgv, in0=lgv,
                    in1=lnm[:].rearrange("p (w o) -> p w o", o=1)
                    .broadcast_to([128, wb, CL]),
                    op=AL.subtract)
                nc.sync.dma_start(
                    out=out_d[w0 * 128:(w0 + wb) * 128, :]
                    .rearrange("(w p) c -> p w c", p=128),
                    in_=lgv)

    nc.compile()
    return nc


def _host_inputs(inputs, cfg, percore):
    x = np.asarray(inputs["x"], np.float32)
    W1 = np.asarray(inputs["W1"], np.float32)
    a_s1 = np.asarray(inputs["a_src1"], np.float32)
    a_d1 = np.asarray(inputs["a_dst1"], np.float32)
    b1 = np.asarray(inputs["b1"], np.float32)
    W2 = np.asarray(inputs["W2"], np.float32)
    a_s2 = np.asarray(inputs["a_src2"], np.float32)
    a_d2 = np.asarray(inputs["a_dst2"], np.float32)
    b2 = np.asarray(inputs["b2"], np.float32)
    H, C1 = cfg["heads"], cfg["hid"]
    D1 = H * C1
    As = np.zeros((D1, H), np.float32)
    Ad = np.zeros((D1, H), np.float32)
    for hd in range(H):
        As[hd * C1:(hd + 1) * C1, hd] = a_s1[hd]
        Ad[hd * C1:(hd + 1) * C1, hd] = a_d1[hd]
    w1cat = np.concatenate([W1, W1 @ As, W1 @ Ad], axis=1)
    wa2s = (W2 @ a_s2[0])[None, :]
    wa2d = (W2 @ a_d2[0])[None, :]
    nsh, npad = cfg["nshard"], cfg["npad"]
    maps = []
    for c in range(cfg["ncores"]):
        xs = x[c * nsh:(c + 1) * nsh]
        xp = np.zeros((npad, cfg["f_in"]), np.float32)
        xp[:xs.shape[0]] = xs
        maps.append(dict(
            x_T=np.ascontiguousarray(xp.T), w1cat=w1cat,
            b1row=b1[None, :], wa2s=wa2s, wa2d=wa2d,
            w2b=W2.astype(BF), b2row=b2[None, :],
            meta_in=percore[c]["meta"], drow_in=percore[c]["drow"],
        ))
    return maps


_CACHE = {}


def kernel(**inputs):
    from concourse import bass_utils

    cfg = FULL_CFG
    ei = np.asarray(inputs["edge_index"])
    src = ei[0].astype(np.int64)
    dst = ei[1].astype(np.int64)

    key = ("full", ei.shape[1])
    if key not in _CACHE:
        st, percore = prep_structure(src, dst, cfg)
        ncobj = build_nc(cfg, st)
        _CACHE[key] = (st, percore, ncobj)
    st, percore, ncobj = _CACHE[key]

    in_maps = _host_inputs(inputs, cfg, percore)
    res = bass_utils.run_bass_kernel_spmd(
        ncobj, in_maps, core_ids=list(range(cfg["ncores"])))
    outs = [res.results[c]["out"][:cfg["nshard"]]
            for c in range(cfg["ncores"])]
    return np.concatenate(outs, axis=0).astype(np.float32)


# revision 37
# speedup vs baseline: 1.3858x; 1.0329x over previous
"""GAT (2-layer, 8-head then 1-head) on 8 Trainium2 NeuronCores.

v2 design: dst-shard nodes across 8 cores (12544-padded shards). Per layer,
each core builds a bf16 node table [npad, 128] = [msg(64) | Es | es | 0...]
where Es = exp(al_src), es = exp(0.2*al_src); AllGathered to all cores.
Attention uses the factorization
    exp(leaky(als+ald)) = max(Es*Ed, es*ed),   Ed = exp(al_dst), ed = exp(.2*al_dst)
so the per-edge work is two multiplies and a max; Ed/ed come from a per-window
resident table looked up on the PE via one-hot S^T matmuls (bf16 hi/lo pairs
accumulated in f32 PSUM for full precision).

Edges are dst-owned, grouped by (block of 7 windows, src-super-shard of 25088
rows, window) with core-common structure; h[src] rows are fetched with ONE
dma_gather per (block, super-shard) stream (~3.9k edges) to amortize the ~5us
SWDGE fixed cost. One-hot S matrices are built on DVE (st1 hits the 2x 16-bit
path against a constant iotacol_wide). Aggregation matmuls accumulate
[msg*alpha | alpha] into per-window PSUM slices packed 7-to-a-bank.
Self-loops are folded in as per-window SBUF-resident rhs computed during the
producer phase and added via one identity matmul per window. Layer 2
aggregates relu1 and applies W2 after aggregation; 1/denominator is applied
after the W2 matmul (row scalars commute).
"""
import sys
import numpy as np

sys.path.insert(0, "/opt/trn_rl_repo")
import ml_dtypes

BF = ml_dtypes.bfloat16

N = 100000
F_IN = 128
HID = 8
HEADS = 8
CLASSES = 40
NEG = 0.2
NC = 8

FULL_CFG = dict(
    ncores=8, nshard=12500, npad=12544, wb=7, ngsh=4, ni_max=3968,
    f_in=128, heads=8, hid=8, classes=40, neg=0.2,
)


def _ceil(a, b):
    return -(-a // b)


# ---------------------------------------------------------------- host prep
def prep_structure(src, dst, cfg):
    """Build core-common call/tile/matmul structure + per-core arrays.

    Returns (st, percore): st holds the shared program structure, percore[c]
    holds the packed meta (idx|dcol) and drow arrays for core c.
    """
    nc_, nsh, npad, wb = cfg["ncores"], cfg["nshard"], cfg["npad"], cfg["wb"]
    ngsh, NI = cfg["ngsh"], cfg["ni_max"]
    nwin = npad // 128                      # 98
    nblk = _ceil(nwin, wb)                  # 14
    assert nwin % wb == 0, "code assumes full blocks"
    gsz = (nc_ * npad) // ngsh              # 25088 rows per gather super-shard
    padrow = 0       # pad slots may gather any row; S/alpha zero them out

    core = dst // nsh
    dstl = dst % nsh
    # Table layout is half-major so each AllGather half is contiguous:
    # global row = half*(NTOT/2) + src_core*(npad/2) + (local_row % (npad/2))
    half_rows = npad // 2                   # 6272
    src_loc = src % nsh
    g_row = ((src_loc // half_rows) * (nc_ * half_rows)
             + (src // nsh) * half_rows + (src_loc % half_rows))
    gsh = g_row // gsz
    gloc = g_row % gsz
    win = dstl // 128
    blk = win // wb

    counts = np.zeros((nc_, nblk, ngsh, nwin), dtype=np.int64)
    np.add.at(counts, (core, blk, gsh, win), 1)
    common = counts.max(axis=0)             # [nblk, ngsh, nwin]
    for b in range(nblk):
        assert common[b, :, b * wb:(b + 1) * wb].min() >= 128, \
            "tile could span >2 windows"

    # per-core edge arrays sorted by (block, gshard, window)
    percore_edges = []
    for c in range(nc_):
        m = core == c
        key = (blk[m].astype(np.int64) * ngsh + gsh[m]) * nwin + win[m]
        o = np.argsort(key, kind="stable")
        percore_edges.append((key[o], gloc[m][o], dstl[m][o], win[m][o]))

    calls = []
    pc_meta = [[] for _ in range(nc_)]
    pc_drow = [[] for _ in range(nc_)]
    win_mms = {}                            # (b, w) -> list of mm ids
    MC = NI // 16 + 32                      # meta cols (idx | dcol-as-i16)

    for b in range(nblk):
        for s in range(ngsh):
            streams = []
            for c in range(nc_):
                kk, sl, dl, wn = percore_edges[c]
                segs = []
                for w in range(b * wb, (b + 1) * wb):
                    kval = (b * ngsh + s) * nwin + w
                    lo = np.searchsorted(kk, kval, "left")
                    hi = np.searchsorted(kk, kval, "right")
                    n_common = common[b, s, w]
                    seg_s = np.full(n_common, padrow, dtype=np.int64)
                    seg_w = np.full(n_common, w, dtype=np.int64)
                    seg_d = np.full(n_common, -1, dtype=np.int64)
                    seg_s[: hi - lo] = sl[lo:hi]
                    seg_w[: hi - lo] = wn[lo:hi]
                    seg_d[: hi - lo] = dl[lo:hi]
                    segs.append(np.stack([seg_s, seg_w, seg_d]))
                streams.append(np.concatenate(segs, axis=1))
            L = streams[0].shape[1]
            pos = 0
            while pos < L:
                ni_real = min(NI, L - pos)
                ni = _ceil(ni_real, 128) * 128
                ntile = ni // 128
                wseg = streams[0][1][pos:pos + ni_real]
                tiles = []
                for t in range(ntile):
                    a, z = t * 128, min((t + 1) * 128, ni_real)
                    if a < ni_real:
                        tw = wseg[a:z]
                        w1 = int(tw.min())
                        assert int(tw.max()) - w1 <= 1, "tile spans >2 windows"
                        straddle = int(tw.max()) > w1
                    else:
                        w1, straddle = int(wseg[-1]), False
                    tiles.append((w1, straddle))
                cid = len(calls)
                mms = []
                for t, (w1, straddle) in enumerate(tiles):
                    for k in ([0, 1] if straddle else [0]):
                        mm_id = (cid, t, k, w1 + k)
                        win_mms.setdefault((b, w1 + k), []).append(mm_id)
                        mms.append(mm_id)
                calls.append(dict(blk=b, shard=s, ni=ni, ntile=ntile,
                                  tiles=tiles, mms=mms))
                for c in range(nc_):
                    ss, ww, dd = streams[c]
                    sl_call = np.full(ni, padrow, dtype=np.int64)
                    rel_call = np.full(ni, 300.0, dtype=np.float64)
                    nreal = min(ni_real, L - pos)
                    sl_call[:nreal] = ss[pos:pos + nreal]
                    for t in range(ntile):
                        a, z = t * 128, min((t + 1) * 128, nreal)
                        if a >= nreal:
                            break
                        w1 = tiles[t][0]
                        dv = dd[pos + a:pos + z]
                        wv = ww[pos + a:pos + z]
                        rel = (wv - w1) * 128 + (dv - wv * 128)
                        rel = np.where(dv < 0, 300.0, rel)
                        rel_call[a:z] = rel
                    # meta: idx wrapped [16, ni/16] tiled x8, then dcol bf16 bits
                    meta = np.zeros((128, MC), np.int16)
                    iw = sl_call.reshape(ni // 16, 16).T.astype(np.int16)
                    meta[:, 0:ni // 16] = np.tile(iw, (8, 1))
                    dcol = rel_call.reshape(ntile, 128).T.astype(BF)
                    meta[:, NI // 16:NI // 16 + ntile] = dcol.view(np.int16)
                    pc_meta[c].append(meta)
                    pc_drow[c].append(rel_call.astype(BF))
                pos += ni_real

    # One PSUM accumulation group per block bank: the first self matmul
    # starts it (start=True zeroes the whole 2KB bank), the absolute last
    # aggregation matmul of the block stops it.
    last_mm_of_blk = {}
    for cl in calls:
        if cl["mms"]:
            last_mm_of_blk[cl["blk"]] = cl["mms"][-1]
    stopset = set(last_mm_of_blk.values())
    assert len(stopset) == nblk, "every block must have edge matmuls"
    for cl in calls:
        cl["flags"] = [(m, m in stopset) for m in cl["mms"]]

    ncalls = len(calls)
    meta_t = [np.zeros((128, MC * ncalls), np.int16) for _ in range(nc_)]
    drow_t = [np.full((1, NI * ncalls), 300.0, BF) for _ in range(nc_)]
    for c in range(nc_):
        for i in range(ncalls):
            meta_t[c][:, i * MC:(i + 1) * MC] = pc_meta[c][i]
            ni = calls[i]["ni"]
            drow_t[c][0, i * NI:i * NI + ni] = pc_drow[c][i]

    st = dict(calls=calls, nwin=nwin, nblk=nblk, ncalls=ncalls, MC=MC)
    percore = [dict(meta=meta_t[c], drow=drow_t[c]) for c in range(nc_)]
    return st, percore


# ---------------------------------------------------------------- program
def build_nc(cfg, st):
    import concourse.bass as bass
    import concourse.bacc as bacc
    import concourse.tile as tile
    import concourse.mybir as mybir
    from concourse.masks import make_identity

    bf16, f32 = mybir.dt.bfloat16, mybir.dt.float32
    i16, i32 = mybir.dt.int16, mybir.dt.int32
    AL = mybir.AluOpType
    AF = mybir.ActivationFunctionType
    ax_x = mybir.AxisListType.X

    nc_, nsh, npad, wb = cfg["ncores"], cfg["nshard"], cfg["npad"], cfg["wb"]
    ngsh, NI = cfg["ngsh"], cfg["ni_max"]
    H, C1, CL = cfg["heads"], cfg["hid"], cfg["classes"]
    D1 = H * C1                      # 64
    NEGS = cfg["neg"]
    nwin, nblk, ncalls = st["nwin"], st["nblk"], st["ncalls"]
    MC = st["MC"]
    NTOT = nc_ * npad
    GS = NTOT // ngsh                # 25088
    ntile_x = npad // 128            # 98
    RH1 = D1 + H                     # 72
    RH2 = D1 + 1                     # 65
    W1C = D1 + 2 * H                 # 80

    nc = bacc.Bacc("TRN2", target_bir_lowering=False, debug=False,
                   enable_asserts=False, num_devices=nc_, num_swdge_queues=4)

    # ---- I/O
    x_T = nc.dram_tensor("x_T", [cfg["f_in"], npad], f32, kind="ExternalInput")
    w1cat = nc.dram_tensor("w1cat", [cfg["f_in"], W1C], f32,
                           kind="ExternalInput")
    b1row = nc.dram_tensor("b1row", [1, D1], f32, kind="ExternalInput")
    wa2s = nc.dram_tensor("wa2s", [1, D1], f32, kind="ExternalInput")
    wa2d = nc.dram_tensor("wa2d", [1, D1], f32, kind="ExternalInput")
    w2b = nc.dram_tensor("w2b", [D1, CL], bf16, kind="ExternalInput")
    b2row = nc.dram_tensor("b2row", [1, CL], f32, kind="ExternalInput")
    meta_in = nc.dram_tensor("meta_in", [128, MC * ncalls], i16,
                             kind="ExternalInput")
    drow_in = nc.dram_tensor("drow_in", [1, NI * ncalls], bf16,
                             kind="ExternalInput")
    out_d = nc.dram_tensor("out", [npad, CL], f32, kind="ExternalOutput")

    with tile.TileContext(nc) as tc:
        with (
            tc.tile_pool(name="const", bufs=1) as cpool,
            tc.tile_pool(name="res", bufs=1) as rp,
            tc.tile_pool(name="p0", bufs=3) as p0,
            tc.tile_pool(name="meta", bufs=6) as mp,
            tc.tile_pool(name="drp", bufs=3) as dpp,
            tc.tile_pool(name="gpool", bufs=5) as gp,
            tc.tile_pool(name="spool", bufs=2) as sp,
            tc.tile_pool(name="rhsp", bufs=3) as rhp,
            tc.tile_pool(name="tv", bufs=3) as tvp,
            tc.tile_pool(name="epi", bufs=2) as ep,
            tc.tile_pool(name="pwin", bufs=2, space="PSUM") as pw,
            tc.tile_pool(name="pald", bufs=2, space="PSUM") as pa,
            tc.tile_pool(name="pm", bufs=2, space="PSUM") as pm,
            tc.tile_pool(name="ptr", bufs=1, space="PSUM") as ptp,
            tc.tile_pool(name="dram", bufs=1, space="DRAM") as dp,
        ):
            # ---------- constants
            ident = cpool.tile([128, 128], f32)
            make_identity(nc, ident[:])
            identb = cpool.tile([128, 128], bf16)
            nc.vector.tensor_copy(identb[:], ident[:])
            iota_i = cpool.tile([128, 128], i32)
            nc.gpsimd.iota(iota_i[:], pattern=[[1, 128]], base=0,
                           channel_multiplier=0)
            iota_mat = cpool.tile([128, 128], bf16)
            nc.vector.tensor_copy(iota_mat[:], iota_i[:])
            iota_mat2 = cpool.tile([128, 128], bf16)
            nc.vector.tensor_scalar_add(iota_mat2[:], iota_mat[:], 128.0)
            ic_i = cpool.tile([128, 1], i32)
            nc.gpsimd.iota(ic_i[:], pattern=[[0, 1]], base=0,
                           channel_multiplier=1)
            iota_col = cpool.tile([128, 1], bf16)
            nc.vector.tensor_copy(iota_col[:], ic_i[:])
            iotacol_w = cpool.tile([128, NI], bf16)
            nc.vector.tensor_copy(iotacol_w[:],
                                  iota_col[:].broadcast_to([128, NI]))
            iotacol2 = cpool.tile([128, 128], bf16)
            nc.vector.tensor_copy(
                iotacol2[:],
                iota_col[:].broadcast_to([128, 128]))
            nc.vector.tensor_scalar_add(iotacol2[:], iotacol2[:], 128.0)
            b1m = cpool.tile([128, D1], f32)
            nc.sync.dma_start(out=b1m[:], in_=b1row[:].to_broadcast([128, D1]))
            wa2sm = cpool.tile([128, D1], f32)
            nc.sync.dma_start(out=wa2sm[:], in_=wa2s[:].to_broadcast([128, D1]))
            wa2dm = cpool.tile([128, D1], f32)
            nc.sync.dma_start(out=wa2dm[:], in_=wa2d[:].to_broadcast([128, D1]))
            b2m = cpool.tile([128, CL], f32)
            nc.sync.dma_start(out=b2m[:], in_=b2row[:].to_broadcast([128, CL]))
            w1c_sb = cpool.tile([cfg["f_in"], W1C], f32)
            nc.sync.dma_start(out=w1c_sb[:], in_=w1cat[:])
            w2b_sb = cpool.tile([D1, CL], bf16)
            nc.sync.dma_start(out=w2b_sb[:], in_=w2b[:])
            clampc = cpool.tile([128, 1], f32)
            nc.vector.memset(clampc[:], 1e30)
            iotat_w = cpool.tile([128, NI], bf16)
            nc.vector.tensor_copy(
                iotat_w[:].rearrange("p (b n) -> p b n", n=128),
                iota_mat[:].rearrange("p (o n) -> p o n", o=1)
                .broadcast_to([128, NI // 128, 128]))

            # resident: Ed/ed window tables (hi/lo bf16) + self-loop rhs
            al1w = rp.tile([128, 32 * nwin], bf16)  # [Edhi8|edhi8|Edlo8|edlo8]
            al2w = rp.tile([128, 4 * nwin], bf16)   # [Edhi|edhi|Edlo|edlo]
            self1 = rp.tile([128, RH1 * nwin], bf16)
            self2 = rp.tile([128, RH2 * nwin], bf16)

            # DRAM tiles (full tables split into window halves so each
            # AllGather half can overlap with producer compute)
            HR = npad // 2               # 6272 rows per half
            t1_own = dp.tile([npad, 128], bf16)
            t1_fa = dp.tile([NTOT // 2, 128], bf16)
            t1_fb = dp.tile([NTOT // 2, 128], bf16)
            t2_own = dp.tile([npad, 128], bf16)
            t2_fa = dp.tile([NTOT // 2, 128], bf16)
            t2_fb = dp.tile([NTOT // 2, 128], bf16)

            # ---------------- P0: produce T1 + al1/self1 tables
            for t in range(ntile_x):
                xt = p0.tile([cfg["f_in"], 128], f32, tag="xt")
                nc.sync.dma_start(out=xt[:], in_=x_T[:, t * 128:(t + 1) * 128])
                ps = pm.tile([128, W1C], f32, space="PSUM", tag="pm")
                nc.tensor.matmul(ps[:], lhsT=xt[:], rhs=w1c_sb[:],
                                 start=True, stop=True)
                t1sb = p0.tile([128, 128], bf16, tag="t1sb")
                nc.vector.tensor_copy(t1sb[:, 0:D1], ps[:, 0:D1])
                nc.scalar.activation(t1sb[:, D1:D1 + H], ps[:, D1:D1 + H],
                                     AF.Exp)
                nc.scalar.activation(t1sb[:, D1 + H:D1 + 2 * H],
                                     ps[:, D1:D1 + H], AF.Exp, scale=NEGS)
                nc.vector.memset(t1sb[:, D1 + 2 * H:128], 0.0)
                Edf = p0.tile([128, H], f32, tag="Edf")
                nc.scalar.activation(Edf[:], ps[:, D1 + H:W1C], AF.Exp)
                edf = p0.tile([128, H], f32, tag="edf")
                nc.scalar.activation(edf[:], ps[:, D1 + H:W1C], AF.Exp,
                                     scale=NEGS)
                o = 32 * t
                nc.vector.tensor_copy(al1w[:, o:o + H], Edf[:])
                nc.vector.tensor_copy(al1w[:, o + H:o + 2 * H], edf[:])
                nc.vector.tensor_tensor(out=al1w[:, o + 16:o + 24],
                                        in0=Edf[:], in1=al1w[:, o:o + H],
                                        op=AL.subtract)
                nc.vector.tensor_tensor(out=al1w[:, o + 24:o + 32],
                                        in0=edf[:], in1=al1w[:, o + H:o + 16],
                                        op=AL.subtract)
                # self-loop alpha + rhs
                av = p0.tile([128, H], f32, tag="av")
                nc.vector.tensor_tensor(out=av[:], in0=t1sb[:, D1:D1 + H],
                                        in1=Edf[:], op=AL.mult)
                bv = p0.tile([128, H], f32, tag="bv")
                nc.vector.tensor_tensor(out=bv[:],
                                        in0=t1sb[:, D1 + H:D1 + 2 * H],
                                        in1=edf[:], op=AL.mult)
                so = RH1 * t
                nc.vector.tensor_tensor(out=self1[:, so + D1:so + RH1],
                                        in0=av[:], in1=bv[:], op=AL.max)
                nc.vector.tensor_tensor(
                    out=self1[:, so:so + D1].rearrange("p (a c) -> p a c",
                                                       c=C1),
                    in0=t1sb[:, 0:D1].rearrange("p (a c) -> p a c", c=C1),
                    in1=self1[:, so + D1:so + RH1]
                    .broadcast_to([128, H, C1]),
                    op=AL.mult)
                nc.sync.dma_start(out=t1_own[t * 128:(t + 1) * 128, :],
                                  in_=t1sb[:])
                if t == ntile_x // 2 - 1:
                    nc.gpsimd.collective_compute(
                        "AllGather", AL.bypass,
                        replica_groups=[list(range(nc_))],
                        ins=[t1_own[0:HR, :].opt()], outs=[t1_fa.opt()],
                    )

            nc.gpsimd.collective_compute(
                "AllGather", AL.bypass,
                replica_groups=[list(range(nc_))],
                ins=[t1_own[HR:npad, :].opt()], outs=[t1_fb.opt()],
            )

            # ---------------- shared edge pass
            def edge_pass(tfa, tfb, alw, selfw, nal, rhw, aws):
                """nal: attention scalars/edge; rhw: rhs width; aws: alw stride."""
                call_i = 0
                calls_by_blk = [[] for _ in range(nblk)]
                for ci, cl in enumerate(st["calls"]):
                    calls_by_blk[cl["blk"]].append(ci)
                for b in range(nblk):
                    ptb = pw.tile([128, wb * RH1], f32, space="PSUM",
                                  tag="pwin")
                    w0 = b * wb
                    for wi in range(wb):
                        w = w0 + wi
                        nc.tensor.matmul(
                            ptb[:, wi * RH1:wi * RH1 + rhw], lhsT=identb[:],
                            rhs=selfw[:, w * rhw:(w + 1) * rhw],
                            start=(wi == 0), stop=False)
                    for ci in calls_by_blk[b]:
                        cl = st["calls"][ci]
                        ni, nt = cl["ni"], cl["ntile"]
                        meta = mp.tile([128, MC], i16, tag="meta")
                        nc.sync.dma_start(out=meta[:],
                                          in_=meta_in[:, ci * MC:(ci + 1) * MC])
                        dcol = meta[:, NI // 16:NI // 16 + nt].bitcast(bf16)
                        drep = dpp.tile([128, NI], bf16, tag="drep")
                        nc.sync.dma_start(
                            out=drep[:, 0:ni],
                            in_=drow_in[:, ci * NI:ci * NI + ni]
                            .to_broadcast([128, ni]))
                        g = gp.tile([128, NI], bf16, tag="g")
                        s_ = cl["shard"]
                        tsrc = (tfa if s_ < 2 else tfb)
                        soff = (s_ % 2) * GS
                        # split the gather across the 4 SWDGE queues so
                        # descriptor generation runs on 4 Q7 pairs in
                        # parallel; chunks <= 1024 idxs so single_packet
                        # stays within the 64-descriptor packet limit
                        tq = _ceil(nt, 4)
                        base = 0
                        for q in range(4):
                            tc_ = min(tq, nt - base)
                            if tc_ <= 0:
                                break
                            niq = tc_ * 128
                            nc.gpsimd.dma_gather(
                                g[:, base * 128:(base + tc_) * 128]
                                .rearrange("p (b e) -> p b e", e=128),
                                tsrc[soff:soff + GS, :],
                                meta[:, base * 8:(base + tc_) * 8],
                                niq, niq, 128,
                                single_packet=True, queue_num=q)
                            base += tc_
                        call_i += 1
                        # one-hot builds (single broadcast operand against a
                        # contiguous tiled-iota constant)
                        s1 = sp.tile([128, NI], bf16, tag="s1")
                        nc.vector.tensor_tensor(
                            out=s1[:, 0:ni].rearrange("p (b n) -> p b n",
                                                      n=128),
                            in0=dcol.broadcast_to([128, nt, 128]),
                            in1=iotat_w[:, 0:ni]
                            .rearrange("p (b n) -> p b n", n=128),
                            op=AL.is_equal)
                        st1 = sp.tile([128, NI], bf16, tag="st1")
                        nc.vector.tensor_tensor(
                            out=st1[:, 0:ni], in0=iotacol_w[:, 0:ni],
                            in1=drep[:, 0:ni], op=AL.is_equal)
                        strads = [t for t, (w1, sdl) in enumerate(cl["tiles"])
                                  if sdl]
                        nstr = len(strads)
                        if nstr:
                            s2s = sp.tile([128, 128 * 8], bf16, tag="s2s")
                            st2s = sp.tile([128, 128 * 8], bf16, tag="st2s")
                            for j, t in enumerate(strads):
                                nc.vector.tensor_tensor(
                                    out=s2s[:, j * 128:(j + 1) * 128],
                                    in0=dcol[:, t:t + 1]
                                    .broadcast_to([128, 128]),
                                    in1=iota_mat2[:], op=AL.is_equal)
                                nc.vector.tensor_tensor(
                                    out=st2s[:, j * 128:(j + 1) * 128],
                                    in0=iotacol2[:],
                                    in1=drep[:, t * 128:(t + 1) * 128],
                                    op=AL.is_equal)
                        sidx = {t: j for j, t in enumerate(strads)}
                        # Ed/ed lookup (hi/lo accumulated in f32 psum)
                        pald = pa.tile([128, 62 * 8], f32, space="PSUM",
                                       tag="pald")
                        na2 = 2 * nal
                        for t, (w1, sdl) in enumerate(cl["tiles"]):
                            po = pald[:, t * na2:(t + 1) * na2]
                            last_t = t == nt - 1
                            nc.tensor.matmul(
                                po, lhsT=st1[:, t * 128:(t + 1) * 128],
                                rhs=alw[:, aws * w1:aws * w1 + na2],
                                start=(t == 0), stop=False)
                            nc.tensor.matmul(
                                po, lhsT=st1[:, t * 128:(t + 1) * 128],
                                rhs=alw[:, aws * w1 + na2:aws * (w1 + 1)],
                                start=False, stop=last_t and not sdl)
                            if sdl:
                                j = sidx[t]
                                sl = st2s[:, j * 128:(j + 1) * 128]
                                nc.tensor.matmul(
                                    po, lhsT=sl,
                                    rhs=alw[:, aws * (w1 + 1):
                                            aws * (w1 + 1) + na2],
                                    start=False, stop=False)
                                nc.tensor.matmul(
                                    po, lhsT=sl,
                                    rhs=alw[:, aws * (w1 + 1) + na2:
                                            aws * (w1 + 2)],
                                    start=False, stop=last_t)
                        # alpha = max(Es*Ed, es*ed); rhs = [msg*alpha | alpha]
                        gv = g[:, 0:ni].rearrange("p (b e) -> p b e", e=128)
                        pv = pald[:, 0:nt * na2].rearrange(
                            "p (b a) -> p b a", a=na2)
                        t1v = tvp.tile([128, 8 * 32], f32, tag="t1v")
                        t2v = tvp.tile([128, 8 * 32], f32, tag="t2v")
                        t1vv = t1v[:, 0:nt * nal].rearrange(
                            "p (b a) -> p b a", a=nal)
                        t2vv = t2v[:, 0:nt * nal].rearrange(
                            "p (b a) -> p b a", a=nal)
                        nc.vector.tensor_tensor(
                            out=t1vv, in0=gv[:, :, D1:D1 + nal],
                            in1=pv[:, :, 0:nal], op=AL.mult)
                        nc.vector.tensor_tensor(
                            out=t2vv, in0=gv[:, :, D1 + nal:D1 + 2 * nal],
                            in1=pv[:, :, nal:na2], op=AL.mult)
                        rhs = rhp.tile([128, RH1 * 32], bf16, tag="rhs")
                        rv = rhs[:, 0:nt * rhw].rearrange(
                            "p (b r) -> p b r", r=rhw)
                        nc.vector.tensor_tensor(
                            out=rv[:, :, D1:D1 + nal], in0=t1vv, in1=t2vv,
                            op=AL.max)
                        cph = D1 // nal
                        nc.vector.tensor_tensor(
                            out=rv[:, :, 0:D1].rearrange(
                                "p b (a c) -> p b a c", c=cph),
                            in0=gv[:, :, 0:D1].rearrange(
                                "p b (a c) -> p b a c", c=cph),
                            in1=rv[:, :, D1:D1 + nal]
                            .broadcast_to([128, nt, nal, cph]),
                            op=AL.mult)
                        # aggregation matmuls
                        for (mm, fstop) in cl["flags"]:
                            _, t, k, w = mm
                            wi = w - w0
                            if k == 0:
                                lhsT = s1[:, t * 128:(t + 1) * 128]
                            else:
                                j = sidx[t]
                                lhsT = s2s[:, j * 128:(j + 1) * 128]
                            nc.tensor.matmul(
                                ptb[:, wi * RH1:wi * RH1 + rhw], lhsT=lhsT,
                                rhs=rhs[:, t * rhw:(t + 1) * rhw],
                                start=False, stop=fstop)
                    yield b, ptb

            # ---------------- L1 pass + epilogue -> T2
            for b, ptb in edge_pass(t1_fa, t1_fb, al1w, self1, H, RH1, 32):
                w0 = b * wb
                ptv = ptb[:].rearrange("p (w r) -> p w r", r=RH1)
                den = ep.tile([128, wb * H], f32, tag="den1")
                nc.vector.tensor_copy(
                    den[:].rearrange("p (w a) -> p w a", a=H),
                    ptv[:, :, D1:RH1])
                rc = ep.tile([128, wb * H], f32, tag="rc1")
                rcv = rc[:].rearrange("p (w a) -> p w a", a=H)
                nc.vector.reciprocal(rc[:], den[:])
                nc.vector.tensor_tensor(
                    out=rc[:], in0=rc[:],
                    in1=clampc[:].broadcast_to([128, wb * H]), op=AL.min)
                o1 = ep.tile([128, wb * D1], f32, tag="o1")
                o1v = o1[:].rearrange("p (w a c) -> p w a c", a=H, c=C1)
                nc.vector.tensor_tensor(
                    out=o1v,
                    in0=ptv[:, :, 0:D1].rearrange("p w (a c) -> p w a c",
                                                  c=C1),
                    in1=rcv.broadcast_to([128, wb, H, C1]), op=AL.mult)
                b1v = b1m[:].rearrange("p (o c) -> p o c", o=1)
                o1w = o1[:].rearrange("p (w c) -> p w c", c=D1)
                nc.vector.tensor_tensor(
                    out=o1w, in0=o1w,
                    in1=b1v.broadcast_to([128, wb, D1]), op=AL.add)
                ra = ep.tile([128, wb * D1], f32, tag="ra")
                nc.scalar.activation(ra[:], o1[:], AF.Relu)
                rav = ra[:].rearrange("p (w c) -> p w c", c=D1)
                t2sb = ep.tile([128, wb * 128], bf16, tag="t2sb")
                t2v_ = t2sb[:].rearrange("p (w c) -> p w c", c=128)
                for wi in range(wb):
                    nc.vector.tensor_copy(
                        t2sb[:, wi * 128:wi * 128 + D1],
                        ra[:, wi * D1:(wi + 1) * D1])
                nc.vector.memset(t2v_[:, :, D1 + 2:128], 0.0)
                tmp = ep.tile([128, wb * D1], f32, tag="altmp")
                tmpv = tmp[:].rearrange("p (w c) -> p w c", c=D1)
                a2 = ep.tile([128, 2 * wb], f32, tag="a2")
                wsv = wa2sm[:].rearrange("p (o c) -> p o c", o=1)
                nc.vector.tensor_tensor(out=tmpv, in0=rav,
                                        in1=wsv.broadcast_to([128, wb, D1]),
                                        op=AL.mult)
                nc.vector.tensor_reduce(
                    a2[:, 0:wb].rearrange("p (w o) -> p w o", o=1), tmpv,
                    axis=ax_x, op=AL.add)
                wdv = wa2dm[:].rearrange("p (o c) -> p o c", o=1)
                nc.vector.tensor_tensor(out=tmpv, in0=rav,
                                        in1=wdv.broadcast_to([128, wb, D1]),
                                        op=AL.mult)
                nc.vector.tensor_reduce(
                    a2[:, wb:2 * wb].rearrange("p (w o) -> p w o", o=1), tmpv,
                    axis=ax_x, op=AL.add)
                # Es2/es2 into table; Ed2/ed2 hi/lo into al2w; self2
                nc.scalar.activation(t2v_[:, :, D1:D1 + 1], a2[:, 0:wb]
                                     .rearrange("p (w o) -> p w o", o=1), AF.Exp)
                nc.scalar.activation(t2v_[:, :, D1 + 1:D1 + 2], a2[:, 0:wb]
                                     .rearrange("p (w o) -> p w o", o=1), AF.Exp,
                                     scale=NEGS)
                Ed2 = ep.tile([128, 2 * wb], f32, tag="Ed2")
                nc.scalar.activation(Ed2[:, 0:wb], a2[:, wb:2 * wb], AF.Exp)
                nc.scalar.activation(Ed2[:, wb:2 * wb], a2[:, wb:2 * wb],
                                     AF.Exp, scale=NEGS)
                awv = al2w[:, 4 * w0:4 * (w0 + wb)].rearrange(
                    "p (w k) -> p w k", k=4)
                E2v = Ed2[:, 0:wb].rearrange("p (w o) -> p w o", o=1)
                e2v = Ed2[:, wb:2 * wb].rearrange("p (w o) -> p w o", o=1)
                nc.vector.tensor_copy(awv[:, :, 0:1], E2v)
                nc.vector.tensor_copy(awv[:, :, 1:2], e2v)
                nc.vector.tensor_tensor(out=awv[:, :, 2:3], in0=E2v,
                                        in1=awv[:, :, 0:1], op=AL.subtract)
                nc.vector.tensor_tensor(out=awv[:, :, 3:4], in0=e2v,
                                        in1=awv[:, :, 1:2], op=AL.subtract)
                sa = ep.tile([128, 2 * wb], f32, tag="sa")
                nc.vector.tensor_tensor(
                    out=sa[:, 0:wb].rearrange("p (w o) -> p w o", o=1),
                    in0=t2v_[:, :, D1:D1 + 1], in1=E2v, op=AL.mult)
                nc.vector.tensor_tensor(
                    out=sa[:, wb:2 * wb].rearrange("p (w o) -> p w o", o=1),
                    in0=t2v_[:, :, D1 + 1:D1 + 2], in1=e2v, op=AL.mult)
                s2v = self2[:, RH2 * w0:RH2 * (w0 + wb)].rearrange(
                    "p (w r) -> p w r", r=RH2)
                nc.vector.tensor_tensor(
                    out=s2v[:, :, D1:RH2],
                    in0=sa[:, 0:wb].rearrange("p (w o) -> p w o", o=1),
                    in1=sa[:, wb:2 * wb].rearrange("p (w o) -> p w o", o=1),
                    op=AL.max)
                nc.vector.tensor_tensor(
                    out=s2v[:, :, 0:D1], in0=t2v_[:, :, 0:D1],
                    in1=s2v[:, :, D1:RH2].broadcast_to([128, wb, D1]),
                    op=AL.mult)
                nc.sync.dma_start(
                    out=t2_own[w0 * 128:(w0 + wb) * 128, :]
                    .rearrange("(w p) c -> p w c", p=128),
                    in_=t2v_)
                if b == nblk // 2 - 1:
                    nc.gpsimd.collective_compute(
                        "AllGather", AL.bypass,
                        replica_groups=[list(range(nc_))],
                        ins=[t2_own[0:HR, :].opt()], outs=[t2_fa.opt()],
                    )

            nc.gpsimd.collective_compute(
                "AllGather", AL.bypass,
                replica_groups=[list(range(nc_))],
                ins=[t2_own[HR:npad, :].opt()], outs=[t2_fb.opt()],
            )

            # ---------------- L2 pass + epilogue -> output
            for b, ptb in edge_pass(t2_fa, t2_fb, al2w, self2, 1, RH2, 4):
                w0 = b * wb
                ptv = ptb[:].rearrange("p (w r) -> p w r", r=RH1)
                den2 = ep.tile([128, wb], f32, tag="den2")
                nc.vector.tensor_copy(
                    den2[:].rearrange("p (w o) -> p w o", o=1),
                    ptv[:, :, D1:D1 + 1])
                rc = ep.tile([128, wb], f32, tag="rc2")
                nc.vector.reciprocal(rc[:], den2[:])
                nc.vector.tensor_tensor(
                    out=rc[:], in0=rc[:],
                    in1=clampc[:].broadcast_to([128, wb]), op=AL.min)
                tca = ep.tile([128, wb * D1], bf16, tag="tca")
                tcav = tca[:].rearrange("p (w c) -> p w c", c=D1)
                nc.vector.tensor_copy(tcav, ptv[:, :, 0:D1])
                lg = ep.tile([128, wb * CL], f32, tag="lg")
                for wi in range(wb):
                    trp = ptp.tile([D1, 128], bf16, space="PSUM", tag="trp")
                    nc.tensor.transpose(out=trp[:],
                                        in_=tca[:, wi * D1:(wi + 1) * D1],
                                        identity=identb[:])
                    trs = ep.tile([D1, 128], bf16, tag="trs")
                    nc.vector.tensor_copy(trs[:], trp[:])
                    op2 = ptp.tile([128, CL], f32, space="PSUM", tag="op2")
                    nc.tensor.matmul(op2[:], lhsT=trs[:], rhs=w2b_sb[:],
                                     start=True, stop=True)
                    nc.vector.scalar_tensor_tensor(
                        out=lg[:, wi * CL:(wi + 1) * CL], in0=op2[:],
                        scalar=rc[:, wi:wi + 1], in1=b2m[:],
                        op0=AL.mult, op1=AL.add)
                lgv = lg[:].rearrange("p (w c) -> p w c", c=CL)
                mx = ep.tile([128, wb], f32, tag="mx")
                mxv = mx[:].rearrange("p (w o) -> p w o", o=1)
                nc.vector.tensor_reduce(mxv, lgv, axis=ax_x, op=AL.max)
                nc.vector.tensor_tensor(out=lgv, in0=lgv,
                                        in1=mxv.broadcast_to([128, wb, CL]),
                                        op=AL.subtract)
                exs = ep.tile([128, wb * CL], f32, tag="exs")
                nc.scalar.activation(exs[:], lg[:], AF.Exp)
                sm = ep.tile([128, wb], f32, tag="sm")
                smv = sm[:].rearrange("p (w o) -> p w o", o=1)
                nc.vector.tensor_reduce(
                    smv, exs[:].rearrange("p (w c) -> p w c", c=CL),
                    axis=ax_x, op=AL.add)
                lnm = ep.tile([128, wb], f32, tag="lnm")
                nc.scalar.activation(lnm[:], sm[:], AF.Ln)
                nc.vector.tensor_tensor(
                    out=lgv, in0=lgv,
                    in1=lnm[:].rearrange("p (w o) -> p w o", o=1)
                    .broadcast_to([128, wb, CL]),
                    op=AL.subtract)
                nc.sync.dma_start(
                    out=out_d[w0 * 128:(w0 + wb) * 128, :]
                    .rearrange("(w p) c -> p w c", p=128),
                    in_=lgv)

    nc.compile()
    return nc


def _host_inputs(inputs, cfg, percore):
    x = np.asarray(inputs["x"], np.float32)
    W1 = np.asarray(inputs["W1"], np.float32)
    a_s1 = np.asarray(inputs["a_src1"], np.float32)
    a_d1 = np.asarray(inputs["a_dst1"], np.float32)
    b1 = np.asarray(inputs["b1"], np.float32)
    W2 = np.asarray(inputs["W2"], np.float32)
    a_s2 = np.asarray(inputs["a_src2"], np.float32)
    a_d2 = np.asarray(inputs["a_dst2"], np.float32)
    b2 = np.asarray(inputs["b2"], np.float32)
    H, C1 = cfg["heads"], cfg["hid"]
    D1 = H * C1
    As = np.zeros((D1, H), np.float32)
    Ad = np.zeros((D1, H), np.float32)
    for hd in range(H):
        As[hd * C1:(hd + 1) * C1, hd] = a_s1[hd]
        Ad[hd * C1:(hd + 1) * C1, hd] = a_d1[hd]
    w1cat = np.concatenate([W1, W1 @ As, W1 @ Ad], axis=1)
    wa2s = (W2 @ a_s2[0])[None, :]
    wa2d = (W2 @ a_d2[0])[None, :]
    nsh, npad = cfg["nshard"], cfg["npad"]
    maps = []
    for c in range(cfg["ncores"]):
        xs = x[c * nsh:(c + 1) * nsh]
        xp = np.zeros((npad, cfg["f_in"]), np.float32)
        xp[:xs.shape[0]] = xs
        maps.append(dict(
            x_T=np.ascontiguousarray(xp.T), w1cat=w1cat,
            b1row=b1[None, :], wa2s=wa2s, wa2d=wa2d,
            w2b=W2.astype(BF), b2row=b2[None, :],
            meta_in=percore[c]["meta"], drow_in=percore[c]["drow"],
        ))
    return maps


_CACHE = {}


def kernel(**inputs):
    from concourse import bass_utils

    cfg = FULL_CFG
    ei = np.asarray(inputs["edge_index"])
    src = ei[0].astype(np.int64)
    dst = ei[1].astype(np.int64)

    key = ("full", ei.shape[1])
    if key not in _CACHE:
        st, percore = prep_structure(src, dst, cfg)
        ncobj = build_nc(cfg, st)
        _CACHE[key] = (st, percore, ncobj)
    st, percore, ncobj = _CACHE[key]

    in_maps = _host_inputs(inputs, cfg, percore)
    res = bass_utils.run_bass_kernel_spmd(
        ncobj, in_maps, core_ids=list(range(cfg["ncores"])))
    outs = [res.results[c]["out"][:cfg["nshard"]]
            for c in range(cfg["ncores"])]
    return np.concatenate(outs, axis=0).astype(np.float32)


# revision 48
# speedup vs baseline: 1.3991x; 1.0096x over previous
"""GAT (2-layer, 8-head then 1-head) on 8 Trainium2 NeuronCores.

v2 design: dst-shard nodes across 8 cores (12544-padded shards). Per layer,
each core builds a bf16 node table [npad, 128] = [msg(64) | Es | es | 0...]
where Es = exp(al_src), es = exp(0.2*al_src); AllGathered to all cores.
Attention uses the factorization
    exp(leaky(als+ald)) = max(Es*Ed, es*ed),   Ed = exp(al_dst), ed = exp(.2*al_dst)
so the per-edge work is two multiplies and a max; Ed/ed come from a per-window
resident table looked up on the PE via one-hot S^T matmuls (bf16 hi/lo pairs
accumulated in f32 PSUM for full precision).

Edges are dst-owned, grouped by (block of 7 windows, src-super-shard of 25088
rows, window) with core-common structure; h[src] rows are fetched with ONE
dma_gather per (block, super-shard) stream (~3.9k edges) to amortize the ~5us
SWDGE fixed cost. One-hot S matrices are built on DVE (st1 hits the 2x 16-bit
path against a constant iotacol_wide). Aggregation matmuls accumulate
[msg*alpha | alpha] into per-window PSUM slices packed 7-to-a-bank.
Self-loops are folded in as per-window SBUF-resident rhs computed during the
producer phase and added via one identity matmul per window. Layer 2
aggregates relu1 and applies W2 after aggregation; 1/denominator is applied
after the W2 matmul (row scalars commute).
"""
import sys
import numpy as np

sys.path.insert(0, "/opt/trn_rl_repo")
import ml_dtypes

BF = ml_dtypes.bfloat16

N = 100000
F_IN = 128
HID = 8
HEADS = 8
CLASSES = 40
NEG = 0.2
NC = 8

FULL_CFG = dict(
    ncores=8, nshard=12500, npad=12544, wb=7, ngsh=4, ni_max=3968,
    f_in=128, heads=8, hid=8, classes=40, neg=0.2,
)


def _ceil(a, b):
    return -(-a // b)


# ---------------------------------------------------------------- host prep
def prep_structure(src, dst, cfg):
    """Build core-common call/tile/matmul structure + per-core arrays.

    Returns (st, percore): st holds the shared program structure, percore[c]
    holds the packed meta (idx|dcol) and drow arrays for core c.
    """
    nc_, nsh, npad, wb = cfg["ncores"], cfg["nshard"], cfg["npad"], cfg["wb"]
    ngsh, NI = cfg["ngsh"], cfg["ni_max"]
    nwin = npad // 128                      # 98
    nblk = _ceil(nwin, wb)                  # 14
    assert nwin % wb == 0, "code assumes full blocks"
    gsz = (nc_ * npad) // ngsh              # 25088 rows per gather super-shard
    padrow = 0       # pad slots may gather any row; S/alpha zero them out

    core = dst // nsh
    dstl = dst % nsh
    # Table layout is half-major so each AllGather half is contiguous:
    # global row = half*(NTOT/2) + src_core*(npad/2) + (local_row % (npad/2))
    half_rows = npad // 2                   # 6272
    src_loc = src % nsh
    g_row = ((src_loc // half_rows) * (nc_ * half_rows)
             + (src // nsh) * half_rows + (src_loc % half_rows))
    gsh = g_row // gsz
    gloc = g_row % gsz
    win = dstl // 128
    blk = win // wb

    counts = np.zeros((nc_, nblk, ngsh, nwin), dtype=np.int64)
    np.add.at(counts, (core, blk, gsh, win), 1)
    common = counts.max(axis=0)             # [nblk, ngsh, nwin]
    for b in range(nblk):
        assert common[b, :, b * wb:(b + 1) * wb].min() >= 128, \
            "tile could span >2 windows"

    # per-core edge arrays sorted by (block, gshard, window)
    percore_edges = []
    for c in range(nc_):
        m = core == c
        key = (blk[m].astype(np.int64) * ngsh + gsh[m]) * nwin + win[m]
        # secondary sort by gather row for DRAM locality within each cell
        o = np.lexsort((gloc[m], key))
        percore_edges.append((key[o], gloc[m][o], dstl[m][o], win[m][o]))

    calls = []
    pc_meta = [[] for _ in range(nc_)]
    pc_drow = [[] for _ in range(nc_)]
    win_mms = {}                            # (b, w) -> list of mm ids
    MC = NI // 16 + 32                      # meta cols (idx | dcol-as-i16)

    for b in range(nblk):
        for s in range(ngsh):
            streams = []
            for c in range(nc_):
                kk, sl, dl, wn = percore_edges[c]
                segs = []
                for w in range(b * wb, (b + 1) * wb):
                    kval = (b * ngsh + s) * nwin + w
                    lo = np.searchsorted(kk, kval, "left")
                    hi = np.searchsorted(kk, kval, "right")
                    n_common = common[b, s, w]
                    seg_s = np.full(n_common, padrow, dtype=np.int64)
                    seg_w = np.full(n_common, w, dtype=np.int64)
                    seg_d = np.full(n_common, -1, dtype=np.int64)
                    seg_s[: hi - lo] = sl[lo:hi]
                    seg_w[: hi - lo] = wn[lo:hi]
                    seg_d[: hi - lo] = dl[lo:hi]
                    segs.append(np.stack([seg_s, seg_w, seg_d]))
                streams.append(np.concatenate(segs, axis=1))
            L = streams[0].shape[1]
            pos = 0
            while pos < L:
                ni_real = min(NI, L - pos)
                ni = _ceil(ni_real, 128) * 128
                ntile = ni // 128
                wseg = streams[0][1][pos:pos + ni_real]
                tiles = []
                for t in range(ntile):
                    a, z = t * 128, min((t + 1) * 128, ni_real)
                    if a < ni_real:
                        tw = wseg[a:z]
                        w1 = int(tw.min())
                        assert int(tw.max()) - w1 <= 1, "tile spans >2 windows"
                        straddle = int(tw.max()) > w1
                    else:
                        w1, straddle = int(wseg[-1]), False
                    tiles.append((w1, straddle))
                cid = len(calls)
                mms = []
                for t, (w1, straddle) in enumerate(tiles):
                    for k in ([0, 1] if straddle else [0]):
                        mm_id = (cid, t, k, w1 + k)
                        win_mms.setdefault((b, w1 + k), []).append(mm_id)
                        mms.append(mm_id)
                strads = [t for t, (w1, sdl) in enumerate(tiles) if sdl]
                assert len(strads) <= 8
                calls.append(dict(blk=b, shard=s, ni=ni, ntile=ntile,
                                  tiles=tiles, mms=mms, strads=strads))
                for c in range(nc_):
                    ss, ww, dd = streams[c]
                    sl_call = np.full(ni, padrow, dtype=np.int64)
                    rel_call = np.full(ni, 300.0, dtype=np.float64)
                    nreal = min(ni_real, L - pos)
                    sl_call[:nreal] = ss[pos:pos + nreal]
                    for t in range(ntile):
                        a, z = t * 128, min((t + 1) * 128, nreal)
                        if a >= nreal:
                            break
                        w1 = tiles[t][0]
                        dv = dd[pos + a:pos + z]
                        wv = ww[pos + a:pos + z]
                        rel = (wv - w1) * 128 + (dv - wv * 128)
                        rel = np.where(dv < 0, 300.0, rel)
                        rel_call[a:z] = rel
                    # meta: idx wrapped [16, ni/16] tiled x8, then dcol bits
                    meta = np.zeros((128, MC), np.int16)
                    iw = sl_call.reshape(ni // 16, 16).T.astype(np.int16)
                    meta[:, 0:ni // 16] = np.tile(iw, (8, 1))
                    dcol = rel_call.reshape(ntile, 128).T.astype(BF)
                    meta[:, NI // 16:NI // 16 + ntile] = dcol.view(np.int16)
                    pc_meta[c].append(meta)
                    pc_drow[c].append(rel_call.astype(BF))
                pos += ni_real

    # One PSUM accumulation group per block bank: the first self matmul
    # starts it (start=True zeroes the whole 2KB bank), the absolute last
    # aggregation matmul of the block stops it.
    last_mm_of_blk = {}
    for cl in calls:
        if cl["mms"]:
            last_mm_of_blk[cl["blk"]] = cl["mms"][-1]
    stopset = set(last_mm_of_blk.values())
    assert len(stopset) == nblk, "every block must have edge matmuls"
    for cl in calls:
        cl["flags"] = [(m, m in stopset) for m in cl["mms"]]

    ncalls = len(calls)
    meta_t = [np.zeros((128, MC * ncalls), np.int16) for _ in range(nc_)]
    drow_t = [np.full((1, NI * ncalls), 300.0, BF) for _ in range(nc_)]
    for c in range(nc_):
        for i in range(ncalls):
            meta_t[c][:, i * MC:(i + 1) * MC] = pc_meta[c][i]
            ni = calls[i]["ni"]
            drow_t[c][0, i * NI:i * NI + ni] = pc_drow[c][i]

    st = dict(calls=calls, nwin=nwin, nblk=nblk, ncalls=ncalls, MC=MC)
    percore = [dict(meta=meta_t[c], drow=drow_t[c]) for c in range(nc_)]
    return st, percore


# ---------------------------------------------------------------- program
def build_nc(cfg, st):
    import concourse.bass as bass
    import concourse.bacc as bacc
    import concourse.tile as tile
    import concourse.mybir as mybir
    from concourse.masks import make_identity

    bf16, f32 = mybir.dt.bfloat16, mybir.dt.float32
    i16, i32 = mybir.dt.int16, mybir.dt.int32
    AL = mybir.AluOpType
    AF = mybir.ActivationFunctionType
    ax_x = mybir.AxisListType.X

    nc_, nsh, npad, wb = cfg["ncores"], cfg["nshard"], cfg["npad"], cfg["wb"]
    ngsh, NI = cfg["ngsh"], cfg["ni_max"]
    H, C1, CL = cfg["heads"], cfg["hid"], cfg["classes"]
    D1 = H * C1                      # 64
    NEGS = cfg["neg"]
    nwin, nblk, ncalls = st["nwin"], st["nblk"], st["ncalls"]
    MC = st["MC"]
    NTOT = nc_ * npad
    GS = NTOT // ngsh                # 25088
    ntile_x = npad // 128            # 98
    RH1 = D1 + H                     # 72
    RH2 = D1 + 1                     # 65
    W1C = D1 + 2 * H                 # 80

    nc = bacc.Bacc("TRN2", target_bir_lowering=False, debug=False,
                   enable_asserts=False, num_devices=nc_, num_swdge_queues=4)

    # ---- I/O
    x_T = nc.dram_tensor("x_T", [cfg["f_in"], npad], f32, kind="ExternalInput")
    w1cat = nc.dram_tensor("w1cat", [cfg["f_in"], W1C], f32,
                           kind="ExternalInput")
    b1row = nc.dram_tensor("b1row", [1, D1], f32, kind="ExternalInput")
    wa2s = nc.dram_tensor("wa2s", [1, D1], f32, kind="ExternalInput")
    wa2d = nc.dram_tensor("wa2d", [1, D1], f32, kind="ExternalInput")
    w2b = nc.dram_tensor("w2b", [D1, CL], bf16, kind="ExternalInput")
    b2row = nc.dram_tensor("b2row", [1, CL], f32, kind="ExternalInput")
    meta_in = nc.dram_tensor("meta_in", [128, MC * ncalls], i16,
                             kind="ExternalInput")
    drow_in = nc.dram_tensor("drow_in", [1, NI * ncalls], bf16,
                             kind="ExternalInput")
    out_d = nc.dram_tensor("out", [npad, CL], f32, kind="ExternalOutput")

    with tile.TileContext(nc) as tc:
        with (
            tc.tile_pool(name="const", bufs=1) as cpool,
            tc.tile_pool(name="res", bufs=1) as rp,
            tc.tile_pool(name="p0", bufs=3) as p0,
            tc.tile_pool(name="meta", bufs=6) as mp,
            tc.tile_pool(name="drp", bufs=3) as dpp,
            tc.tile_pool(name="spool", bufs=2) as sp,
            tc.tile_pool(name="gpool", bufs=5) as gp,
            tc.tile_pool(name="rhsp", bufs=3) as rhp,
            tc.tile_pool(name="tv", bufs=3) as tvp,
            tc.tile_pool(name="epi", bufs=2) as ep,
            tc.tile_pool(name="pwin", bufs=2, space="PSUM") as pw,
            tc.tile_pool(name="pald", bufs=2, space="PSUM") as pa,
            tc.tile_pool(name="pm", bufs=2, space="PSUM") as pm,
            tc.tile_pool(name="ptr", bufs=1, space="PSUM") as ptp,
            tc.tile_pool(name="dram", bufs=1, space="DRAM") as dp,
        ):
            # ---------- constants
            ident = cpool.tile([128, 128], f32)
            make_identity(nc, ident[:])
            identb = cpool.tile([128, 128], bf16)
            nc.vector.tensor_copy(identb[:], ident[:])
            iota_i = cpool.tile([128, 128], i32)
            nc.gpsimd.iota(iota_i[:], pattern=[[1, 128]], base=0,
                           channel_multiplier=0)
            iota_mat = cpool.tile([128, 128], bf16)
            nc.vector.tensor_copy(iota_mat[:], iota_i[:])
            iota_mat2 = cpool.tile([128, 128], bf16)
            nc.vector.tensor_scalar_add(iota_mat2[:], iota_mat[:], 128.0)
            ic_i = cpool.tile([128, 1], i32)
            nc.gpsimd.iota(ic_i[:], pattern=[[0, 1]], base=0,
                           channel_multiplier=1)
            iota_col = cpool.tile([128, 1], bf16)
            nc.vector.tensor_copy(iota_col[:], ic_i[:])
            iotacol_w = cpool.tile([128, NI], bf16)
            nc.vector.tensor_copy(iotacol_w[:],
                                  iota_col[:].broadcast_to([128, NI]))
            iotacol2 = cpool.tile([128, 128], bf16)
            nc.vector.tensor_copy(
                iotacol2[:],
                iota_col[:].broadcast_to([128, 128]))
            nc.vector.tensor_scalar_add(iotacol2[:], iotacol2[:], 128.0)
            iotat_w = cpool.tile([128, NI], bf16)
            nc.vector.tensor_copy(
                iotat_w[:].rearrange("p (b n) -> p b n", n=128),
                iota_mat[:].rearrange("p (o n) -> p o n", o=1)
                .broadcast_to([128, NI // 128, 128]))
            b1m = cpool.tile([128, D1], f32)
            nc.sync.dma_start(out=b1m[:], in_=b1row[:].to_broadcast([128, D1]))
            wa2sm = cpool.tile([128, D1], f32)
            nc.sync.dma_start(out=wa2sm[:], in_=wa2s[:].to_broadcast([128, D1]))
            wa2dm = cpool.tile([128, D1], f32)
            nc.sync.dma_start(out=wa2dm[:], in_=wa2d[:].to_broadcast([128, D1]))
            b2m = cpool.tile([128, CL], f32)
            nc.sync.dma_start(out=b2m[:], in_=b2row[:].to_broadcast([128, CL]))
            w1c_sb = cpool.tile([cfg["f_in"], W1C], f32)
            nc.sync.dma_start(out=w1c_sb[:], in_=w1cat[:])
            w2b_sb = cpool.tile([D1, CL], bf16)
            nc.sync.dma_start(out=w2b_sb[:], in_=w2b[:])
            clampc = cpool.tile([128, 1], f32)
            nc.vector.memset(clampc[:], 1e30)

            # resident: Ed/ed window tables (hi/lo bf16) + self-loop rhs
            al1w = rp.tile([128, 32 * nwin], bf16)  # [Edhi8|edhi8|Edlo8|edlo8]
            al2w = rp.tile([128, 4 * nwin], bf16)   # [Edhi|edhi|Edlo|edlo]
            self1 = rp.tile([128, RH1 * nwin], bf16)
            self2 = rp.tile([128, RH2 * nwin], bf16)

            # DRAM tiles (full tables split into window halves so each
            # AllGather half can overlap with producer compute)
            HR = npad // 2               # 6272 rows per half
            t1_own = dp.tile([npad, 128], bf16)
            t1_fa = dp.tile([NTOT // 2, 128], bf16)
            t1_fb = dp.tile([NTOT // 2, 128], bf16)
            t2_own = dp.tile([npad, 128], bf16)
            t2_fa = dp.tile([NTOT // 2, 128], bf16)
            t2_fb = dp.tile([NTOT // 2, 128], bf16)

            # ---------------- P0: produce T1 + al1/self1 tables
            for t in range(ntile_x):
                xt = p0.tile([cfg["f_in"], 128], f32, tag="xt")
                nc.sync.dma_start(out=xt[:], in_=x_T[:, t * 128:(t + 1) * 128])
                ps = pm.tile([128, W1C], f32, space="PSUM", tag="pm")
                nc.tensor.matmul(ps[:], lhsT=xt[:], rhs=w1c_sb[:],
                                 start=True, stop=True)
                t1sb = p0.tile([128, 128], bf16, tag="t1sb")
                nc.vector.tensor_copy(t1sb[:, 0:D1], ps[:, 0:D1])
                nc.scalar.activation(t1sb[:, D1:D1 + H], ps[:, D1:D1 + H],
                                     AF.Exp)
                nc.scalar.activation(t1sb[:, D1 + H:D1 + 2 * H],
                                     ps[:, D1:D1 + H], AF.Exp, scale=NEGS)
                nc.vector.memset(t1sb[:, D1 + 2 * H:128], 0.0)
                Edf = p0.tile([128, H], f32, tag="Edf")
                nc.scalar.activation(Edf[:], ps[:, D1 + H:W1C], AF.Exp)
                edf = p0.tile([128, H], f32, tag="edf")
                nc.scalar.activation(edf[:], ps[:, D1 + H:W1C], AF.Exp,
                                     scale=NEGS)
                o = 32 * t
                nc.vector.tensor_copy(al1w[:, o:o + H], Edf[:])
                nc.vector.tensor_copy(al1w[:, o + H:o + 2 * H], edf[:])
                nc.vector.tensor_tensor(out=al1w[:, o + 16:o + 24],
                                        in0=Edf[:], in1=al1w[:, o:o + H],
                                        op=AL.subtract)
                nc.vector.tensor_tensor(out=al1w[:, o + 24:o + 32],
                                        in0=edf[:], in1=al1w[:, o + H:o + 16],
                                        op=AL.subtract)
                # self-loop alpha + rhs
                av = p0.tile([128, H], f32, tag="av")
                nc.vector.tensor_tensor(out=av[:], in0=t1sb[:, D1:D1 + H],
                                        in1=Edf[:], op=AL.mult)
                bv = p0.tile([128, H], f32, tag="bv")
                nc.vector.tensor_tensor(out=bv[:],
                                        in0=t1sb[:, D1 + H:D1 + 2 * H],
                                        in1=edf[:], op=AL.mult)
                so = RH1 * t
                nc.vector.tensor_tensor(out=self1[:, so + D1:so + RH1],
                                        in0=av[:], in1=bv[:], op=AL.max)
                nc.vector.tensor_tensor(
                    out=self1[:, so:so + D1].rearrange("p (a c) -> p a c",
                                                       c=C1),
                    in0=t1sb[:, 0:D1].rearrange("p (a c) -> p a c", c=C1),
                    in1=self1[:, so + D1:so + RH1]
                    .broadcast_to([128, H, C1]),
                    op=AL.mult)
                nc.sync.dma_start(out=t1_own[t * 128:(t + 1) * 128, :],
                                  in_=t1sb[:])
                if t == ntile_x // 2 - 1:
                    nc.gpsimd.collective_compute(
                        "AllGather", AL.bypass,
                        replica_groups=[list(range(nc_))],
                        ins=[t1_own[0:HR, :].opt()], outs=[t1_fa.opt()],
                    )

            nc.gpsimd.collective_compute(
                "AllGather", AL.bypass,
                replica_groups=[list(range(nc_))],
                ins=[t1_own[HR:npad, :].opt()], outs=[t1_fb.opt()],
            )

            # ---------------- shared edge pass
            def edge_pass(tfa, tfb, alw, selfw, nal, rhw, aws):
                """nal: attention scalars/edge; rhw: rhs width; aws: alw stride."""
                call_i = 0
                calls_by_blk = [[] for _ in range(nblk)]
                for ci, cl in enumerate(st["calls"]):
                    calls_by_blk[cl["blk"]].append(ci)
                for b in range(nblk):
                    ptb = pw.tile([128, wb * RH1], f32, space="PSUM",
                                  tag="pwin")
                    w0 = b * wb
                    for wi in range(wb):
                        w = w0 + wi
                        nc.tensor.matmul(
                            ptb[:, wi * RH1:wi * RH1 + rhw], lhsT=identb[:],
                            rhs=selfw[:, w * rhw:(w + 1) * rhw],
                            start=(wi == 0), stop=False)
                    for ci in calls_by_blk[b]:
                        cl = st["calls"][ci]
                        ni, nt = cl["ni"], cl["ntile"]
                        meta = mp.tile([128, MC], i16, tag="meta")
                        nc.sync.dma_start(out=meta[:],
                                          in_=meta_in[:, ci * MC:(ci + 1) * MC])
                        dcol = meta[:, NI // 16:NI // 16 + nt].bitcast(bf16)
                        drep = dpp.tile([128, NI], bf16, tag="drep")
                        nc.sync.dma_start(
                            out=drep[:, 0:ni],
                            in_=drow_in[:, ci * NI:ci * NI + ni]
                            .to_broadcast([128, ni]))
                        g = gp.tile([128, NI], bf16, tag="g")
                        s_ = cl["shard"]
                        tsrc = (tfa if s_ < 2 else tfb)
                        soff = (s_ % 2) * GS
                        # split the gather across the 4 SWDGE queues so
                        # descriptor generation runs on 4 Q7 pairs in
                        # parallel; chunks <= 1024 idxs so single_packet
                        # stays within the 64-descriptor packet limit
                        tq = _ceil(nt, 4)
                        base = 0
                        for q in range(4):
                            tc_ = min(tq, nt - base)
                            if tc_ <= 0:
                                break
                            niq = tc_ * 128
                            nc.gpsimd.dma_gather(
                                g[:, base * 128:(base + tc_) * 128]
                                .rearrange("p (b e) -> p b e", e=128),
                                tsrc[soff:soff + GS, :],
                                meta[:, base * 8:(base + tc_) * 8],
                                niq, niq, 128,
                                single_packet=True, queue_num=q)
                            base += tc_
                        call_i += 1
                        # one-hot builds
                        s1 = sp.tile([128, NI], bf16, tag="s1")
                        nc.vector.tensor_tensor(
                            out=s1[:, 0:ni].rearrange("p (b n) -> p b n",
                                                      n=128),
                            in0=dcol.broadcast_to([128, nt, 128]),
                            in1=iotat_w[:, 0:ni]
                            .rearrange("p (b n) -> p b n", n=128),
                            op=AL.is_equal)
                        st1 = sp.tile([128, NI], bf16, tag="st1")
                        nc.vector.tensor_tensor(
                            out=st1[:, 0:ni], in0=iotacol_w[:, 0:ni],
                            in1=drep[:, 0:ni], op=AL.is_equal)
                        strads = cl["strads"]
                        if strads:
                            s2s = sp.tile([128, 128 * 8], bf16, tag="s2s")
                            st2s = sp.tile([128, 128 * 8], bf16, tag="st2s")
                            for j, t in enumerate(strads):
                                nc.vector.tensor_tensor(
                                    out=s2s[:, j * 128:(j + 1) * 128],
                                    in0=dcol[:, t:t + 1]
                                    .broadcast_to([128, 128]),
                                    in1=iota_mat2[:], op=AL.is_equal)
                                nc.vector.tensor_tensor(
                                    out=st2s[:, j * 128:(j + 1) * 128],
                                    in0=iotacol2[:],
                                    in1=drep[:, t * 128:(t + 1) * 128],
                                    op=AL.is_equal)
                        sidx = {t: j for j, t in enumerate(strads)}
                        # Ed/ed lookup (hi/lo accumulated in f32 psum)
                        pald = pa.tile([128, 62 * 8], f32, space="PSUM",
                                       tag="pald")
                        na2 = 2 * nal
                        for t, (w1, sdl) in enumerate(cl["tiles"]):
                            po = pald[:, t * na2:(t + 1) * na2]
                            last_t = t == nt - 1
                            nc.tensor.matmul(
                                po, lhsT=st1[:, t * 128:(t + 1) * 128],
                                rhs=alw[:, aws * w1:aws * w1 + na2],
                                start=(t == 0), stop=False)
                            nc.tensor.matmul(
                                po, lhsT=st1[:, t * 128:(t + 1) * 128],
                                rhs=alw[:, aws * w1 + na2:aws * (w1 + 1)],
                                start=False, stop=last_t and not sdl)
                            if sdl:
                                j = sidx[t]
                                sl = st2s[:, j * 128:(j + 1) * 128]
                                nc.tensor.matmul(
                                    po, lhsT=sl,
                                    rhs=alw[:, aws * (w1 + 1):
                                            aws * (w1 + 1) + na2],
                                    start=False, stop=False)
                                nc.tensor.matmul(
                                    po, lhsT=sl,
                                    rhs=alw[:, aws * (w1 + 1) + na2:
                                            aws * (w1 + 2)],
                                    start=False, stop=last_t)
                        # alpha = max(Es*Ed, es*ed); rhs = [msg*alpha | alpha]
                        gv = g[:, 0:ni].rearrange("p (b e) -> p b e", e=128)
                        pv = pald[:, 0:nt * na2].rearrange(
                            "p (b a) -> p b a", a=na2)
                        t1v = tvp.tile([128, 16 * 32], f32, tag="t1v")
                        tvv = t1v[:, 0:nt * na2].rearrange(
                            "p (b a) -> p b a", a=na2)
                        nc.vector.tensor_tensor(
                            out=tvv, in0=gv[:, :, D1:D1 + na2],
                            in1=pv[:, :, 0:na2], op=AL.mult)
                        t1vv = tvv[:, :, 0:nal]
                        t2vv = tvv[:, :, nal:na2]
                        rhs = rhp.tile([128, RH1 * 32], bf16, tag="rhs")
                        rv = rhs[:, 0:nt * rhw].rearrange(
                            "p (b r) -> p b r", r=rhw)
                        nc.vector.tensor_tensor(
                            out=rv[:, :, D1:D1 + nal], in0=t1vv, in1=t2vv,
                            op=AL.max)
                        cph = D1 // nal
                        nc.vector.tensor_tensor(
                            out=rv[:, :, 0:D1].rearrange(
                                "p b (a c) -> p b a c", c=cph),
                            in0=gv[:, :, 0:D1].rearrange(
                                "p b (a c) -> p b a c", c=cph),
                            in1=rv[:, :, D1:D1 + nal]
                            .broadcast_to([128, nt, nal, cph]),
                            op=AL.mult)
                        # aggregation matmuls
                        for (mm, fstop) in cl["flags"]:
                            _, t, k, w = mm
                            wi = w - w0
                            if k == 0:
                                lhsT = s1[:, t * 128:(t + 1) * 128]
                            else:
                                j = sidx[t]
                                lhsT = s2s[:, j * 128:(j + 1) * 128]
                            nc.tensor.matmul(
                                ptb[:, wi * RH1:wi * RH1 + rhw], lhsT=lhsT,
                                rhs=rhs[:, t * rhw:(t + 1) * rhw],
                                start=False, stop=fstop)
                    yield b, ptb

            # ---------------- L1 pass + epilogue -> T2
            for b, ptb in edge_pass(t1_fa, t1_fb, al1w, self1, H, RH1, 32):
                w0 = b * wb
                ptv = ptb[:].rearrange("p (w r) -> p w r", r=RH1)
                den = ep.tile([128, wb * H], f32, tag="den1")
                nc.vector.tensor_copy(
                    den[:].rearrange("p (w a) -> p w a", a=H),
                    ptv[:, :, D1:RH1])
                rc = ep.tile([128, wb * H], f32, tag="rc1")
                rcv = rc[:].rearrange("p (w a) -> p w a", a=H)
                nc.vector.reciprocal(rc[:], den[:])
                nc.vector.tensor_tensor(
                    out=rc[:], in0=rc[:],
                    in1=clampc[:].broadcast_to([128, wb * H]), op=AL.min)
                o1 = ep.tile([128, wb * D1], f32, tag="o1")
                o1v = o1[:].rearrange("p (w a c) -> p w a c", a=H, c=C1)
                nc.vector.tensor_tensor(
                    out=o1v,
                    in0=ptv[:, :, 0:D1].rearrange("p w (a c) -> p w a c",
                                                  c=C1),
                    in1=rcv.broadcast_to([128, wb, H, C1]), op=AL.mult)
                b1v = b1m[:].rearrange("p (o c) -> p o c", o=1)
                o1w = o1[:].rearrange("p (w c) -> p w c", c=D1)
                nc.vector.tensor_tensor(
                    out=o1w, in0=o1w,
                    in1=b1v.broadcast_to([128, wb, D1]), op=AL.add)
                ra = ep.tile([128, wb * D1], f32, tag="ra")
                nc.scalar.activation(ra[:], o1[:], AF.Relu)
                rav = ra[:].rearrange("p (w c) -> p w c", c=D1)
                t2sb = ep.tile([128, wb * 128], bf16, tag="t2sb")
                t2v_ = t2sb[:].rearrange("p (w c) -> p w c", c=128)
                for wi in range(wb):
                    nc.vector.tensor_copy(
                        t2sb[:, wi * 128:wi * 128 + D1],
                        ra[:, wi * D1:(wi + 1) * D1])
                nc.vector.memset(t2v_[:, :, D1 + 2:128], 0.0)
                tmp = ep.tile([128, wb * D1], f32, tag="altmp")
                tmpv = tmp[:].rearrange("p (w c) -> p w c", c=D1)
                a2 = ep.tile([128, 2 * wb], f32, tag="a2")
                wsv = wa2sm[:].rearrange("p (o c) -> p o c", o=1)
                nc.vector.tensor_tensor(out=tmpv, in0=rav,
                                        in1=wsv.broadcast_to([128, wb, D1]),
                                        op=AL.mult)
                nc.vector.tensor_reduce(
                    a2[:, 0:wb].rearrange("p (w o) -> p w o", o=1), tmpv,
                    axis=ax_x, op=AL.add)
                wdv = wa2dm[:].rearrange("p (o c) -> p o c", o=1)
                nc.vector.tensor_tensor(out=tmpv, in0=rav,
                                        in1=wdv.broadcast_to([128, wb, D1]),
                                        op=AL.mult)
                nc.vector.tensor_reduce(
                    a2[:, wb:2 * wb].rearrange("p (w o) -> p w o", o=1), tmpv,
                    axis=ax_x, op=AL.add)
                # Es2/es2 into table; Ed2/ed2 hi/lo into al2w; self2
                nc.scalar.activation(t2v_[:, :, D1:D1 + 1], a2[:, 0:wb]
                                     .rearrange("p (w o) -> p w o", o=1), AF.Exp)
                nc.scalar.activation(t2v_[:, :, D1 + 1:D1 + 2], a2[:, 0:wb]
                                     .rearrange("p (w o) -> p w o", o=1), AF.Exp,
                                     scale=NEGS)
                Ed2 = ep.tile([128, 2 * wb], f32, tag="Ed2")
                nc.scalar.activation(Ed2[:, 0:wb], a2[:, wb:2 * wb], AF.Exp)
                nc.scalar.activation(Ed2[:, wb:2 * wb], a2[:, wb:2 * wb],
                                     AF.Exp, scale=NEGS)
                awv = al2w[:, 4 * w0:4 * (w0 + wb)].rearrange(
                    "p (w k) -> p w k", k=4)
                E2v = Ed2[:, 0:wb].rearrange("p (w o) -> p w o", o=1)
                e2v = Ed2[:, wb:2 * wb].rearrange("p (w o) -> p w o", o=1)
                nc.vector.tensor_copy(awv[:, :, 0:1], E2v)
                nc.vector.tensor_copy(awv[:, :, 1:2], e2v)
                nc.vector.tensor_tensor(out=awv[:, :, 2:3], in0=E2v,
                                        in1=awv[:, :, 0:1], op=AL.subtract)
                nc.vector.tensor_tensor(out=awv[:, :, 3:4], in0=e2v,
                                        in1=awv[:, :, 1:2], op=AL.subtract)
                sa = ep.tile([128, 2 * wb], f32, tag="sa")
                nc.vector.tensor_tensor(
                    out=sa[:, 0:wb].rearrange("p (w o) -> p w o", o=1),
                    in0=t2v_[:, :, D1:D1 + 1], in1=E2v, op=AL.mult)
                nc.vector.tensor_tensor(
                    out=sa[:, wb:2 * wb].rearrange("p (w o) -> p w o", o=1),
                    in0=t2v_[:, :, D1 + 1:D1 + 2], in1=e2v, op=AL.mult)
                s2v = self2[:, RH2 * w0:RH2 * (w0 + wb)].rearrange(
                    "p (w r) -> p w r", r=RH2)
                nc.vector.tensor_tensor(
                    out=s2v[:, :, D1:RH2],
                    in0=sa[:, 0:wb].rearrange("p (w o) -> p w o", o=1),
                    in1=sa[:, wb:2 * wb].rearrange("p (w o) -> p w o", o=1),
                    op=AL.max)
                nc.vector.tensor_tensor(
                    out=s2v[:, :, 0:D1], in0=t2v_[:, :, 0:D1],
                    in1=s2v[:, :, D1:RH2].broadcast_to([128, wb, D1]),
                    op=AL.mult)
                nc.sync.dma_start(
                    out=t2_own[w0 * 128:(w0 + wb) * 128, :]
                    .rearrange("(w p) c -> p w c", p=128),
                    in_=t2v_)
                if b == nblk // 2 - 1:
                    nc.gpsimd.collective_compute(
                        "AllGather", AL.bypass,
                        replica_groups=[list(range(nc_))],
                        ins=[t2_own[0:HR, :].opt()], outs=[t2_fa.opt()],
                    )

            nc.gpsimd.collective_compute(
                "AllGather", AL.bypass,
                replica_groups=[list(range(nc_))],
                ins=[t2_own[HR:npad, :].opt()], outs=[t2_fb.opt()],
            )

            # ---------------- L2 pass + epilogue -> output
            for b, ptb in edge_pass(t2_fa, t2_fb, al2w, self2, 1, RH2, 4):
                w0 = b * wb
                ptv = ptb[:].rearrange("p (w r) -> p w r", r=RH1)
                den2 = ep.tile([128, wb], f32, tag="den2")
                nc.vector.tensor_copy(
                    den2[:].rearrange("p (w o) -> p w o", o=1),
                    ptv[:, :, D1:D1 + 1])
                rc = ep.tile([128, wb], f32, tag="rc2")
                nc.vector.reciprocal(rc[:], den2[:])
                nc.vector.tensor_tensor(
                    out=rc[:], in0=rc[:],
                    in1=clampc[:].broadcast_to([128, wb]), op=AL.min)
                tca = ep.tile([128, wb * D1], bf16, tag="tca")
                tcav = tca[:].rearrange("p (w c) -> p w c", c=D1)
                nc.vector.tensor_copy(tcav, ptv[:, :, 0:D1])
                lg = ep.tile([128, wb * CL], f32, tag="lg")
                for wi in range(wb):
                    trp = ptp.tile([D1, 128], bf16, space="PSUM", tag="trp")
                    nc.tensor.transpose(out=trp[:],
                                        in_=tca[:, wi * D1:(wi + 1) * D1],
                                        identity=identb[:])
                    trs = ep.tile([D1, 128], bf16, tag="trs")
                    nc.vector.tensor_copy(trs[:], trp[:])
                    op2 = ptp.tile([128, CL], f32, space="PSUM", tag="op2")
                    nc.tensor.matmul(op2[:], lhsT=trs[:], rhs=w2b_sb[:],
                                     start=True, stop=True)
                    nc.vector.scalar_tensor_tensor(
                        out=lg[:, wi * CL:(wi + 1) * CL], in0=op2[:],
                        scalar=rc[:, wi:wi + 1], in1=b2m[:],
                        op0=AL.mult, op1=AL.add)
                lgv = lg[:].rearrange("p (w c) -> p w c", c=CL)
                mx = ep.tile([128, wb], f32, tag="mx")
                mxv = mx[:].rearrange("p (w o) -> p w o", o=1)
                nc.vector.tensor_reduce(mxv, lgv, axis=ax_x, op=AL.max)
                nc.vector.tensor_tensor(out=lgv, in0=lgv,
                                        in1=mxv.broadcast_to([128, wb, CL]),
                                        op=AL.subtract)
                exs = ep.tile([128, wb * CL], f32, tag="exs")
                nc.scalar.activation(exs[:], lg[:], AF.Exp)
                sm = ep.tile([128, wb], f32, tag="sm")
                smv = sm[:].rearrange("p (w o) -> p w o", o=1)
                nc.vector.tensor_reduce(
                    smv, exs[:].rearrange("p (w c) -> p w c", c=CL),
                    axis=ax_x, op=AL.add)
                lnm = ep.tile([128, wb], f32, tag="lnm")
                nc.scalar.activation(lnm[:], sm[:], AF.Ln)
                nc.vector.tensor_tensor(
                    out=lgv, in0=lgv,
                    in1=lnm[:].rearrange("p (w o) -> p w o", o=1)
                    .broadcast_to([128, wb, CL]),
                    op=AL.subtract)
                nc.sync.dma_start(
                    out=out_d[w0 * 128:(w0 + wb) * 128, :]
                    .rearrange("(w p) c -> p w c", p=128),
                    in_=lgv)

    nc.compile()
    return nc


def _host_inputs(inputs, cfg, percore):
    x = np.asarray(inputs["x"], np.float32)
    W1 = np.asarray(inputs["W1"], np.float32)
    a_s1 = np.asarray(inputs["a_src1"], np.float32)
    a_d1 = np.asarray(inputs["a_dst1"], np.float32)
    b1 = np.asarray(inputs["b1"], np.float32)
    W2 = np.asarray(inputs["W2"], np.float32)
    a_s2 = np.asarray(inputs["a_src2"], np.float32)
    a_d2 = np.asarray(inputs["a_dst2"], np.float32)
    b2 = np.asarray(inputs["b2"], np.float32)
    H, C1 = cfg["heads"], cfg["hid"]
    D1 = H * C1
    As = np.zeros((D1, H), np.float32)
    Ad = np.zeros((D1, H), np.float32)
    for hd in range(H):
        As[hd * C1:(hd + 1) * C1, hd] = a_s1[hd]
        Ad[hd * C1:(hd + 1) * C1, hd] = a_d1[hd]
    w1cat = np.concatenate([W1, W1 @ As, W1 @ Ad], axis=1)
    wa2s = (W2 @ a_s2[0])[None, :]
    wa2d = (W2 @ a_d2[0])[None, :]
    nsh, npad = cfg["nshard"], cfg["npad"]
    maps = []
    for c in range(cfg["ncores"]):
        xs = x[c * nsh:(c + 1) * nsh]
        xp = np.zeros((npad, cfg["f_in"]), np.float32)
        xp[:xs.shape[0]] = xs
        maps.append(dict(
            x_T=np.ascontiguousarray(xp.T), w1cat=w1cat,
            b1row=b1[None, :], wa2s=wa2s, wa2d=wa2d,
            w2b=W2.astype(BF), b2row=b2[None, :],
            meta_in=percore[c]["meta"], drow_in=percore[c]["drow"],
        ))
    return maps


_CACHE = {}


def kernel(**inputs):
    from concourse import bass_utils

    cfg = FULL_CFG
    ei = np.asarray(inputs["edge_index"])
    src = ei[0].astype(np.int64)
    dst = ei[1].astype(np.int64)

    key = ("full", ei.shape[1])
    if key not in _CACHE:
        st, percore = prep_structure(src, dst, cfg)
        ncobj = build_nc(cfg, st)
        _CACHE[key] = (st, percore, ncobj)
    st, percore, ncobj = _CACHE[key]

    in_maps = _host_inputs(inputs, cfg, percore)
    res = bass_utils.run_bass_kernel_spmd(
        ncobj, in_maps, core_ids=list(range(cfg["ncores"])))
    outs = [res.results[c]["out"][:cfg["nshard"]]
            for c in range(cfg["ncores"])]
    return np.concatenate(outs, axis=0).astype(np.float32)
